# revision 36
# baseline (speedup 1.0000x reference)
"""Trainium2 Bass kernel for the DreamWorld dense-CNN model.

Contract: kernel(**inputs) takes the FULL unsharded numpy inputs (as produced
by the reference setup_inputs) and returns the full outputs. Internally the
batch is sharded across 8 NeuronCores (pure data parallel); the small weights
are replicated. All shapes/strategy are hardcoded.
"""

import numpy as np

import concourse.bacc as bacc
import concourse.bass as bass
import concourse.mybir as mybir
import concourse.tile as tile
from concourse.bass_utils import run_bass_kernel_spmd

N_PIECES = 4
N_ACTIONS = 4
H = W = 4
NPIX = H * W
B_TOTAL = 131072
NCORES = 8
BC = B_TOTAL // NCORES          # per-core batch
TILE_N = 512                    # samples per device tile
NT = BC // TILE_N               # tiles per core
NCHUNK = TILE_N // 128          # 128-sample chunks per tile

F32 = mybir.dt.float32
BF16 = mybir.dt.bfloat16
F16 = mybir.dt.float16
I32 = mybir.dt.int32

# matmul/activation compute dtype for the hidden layers ("float32",
# "bfloat16"); outputs always produced/stored in f32.
MM_DTYPE = "bfloat16"


# ---------------------------------------------------------------- host math
def _build_weight_mats(p):
    """Transform reference conv/fc weights into the dense feature-major
    matmul operands used on device. All f32 numpy."""
    w_start1 = p["w_start1"]  # [32, 8, 3, 3]
    b_start1 = p["b_start1"]
    w_start2 = p["w_start2"][:, :, 0, 0]  # [16, 32]
    b_start2 = p["b_start2"]
    w_ob1 = p["w_ob1"][:, :, 0, 0]  # [16, 16]
    b_ob1 = p["b_ob1"]
    w_ob2 = p["w_ob2"][:, :, 0, 0]  # [8, 20]
    b_ob2 = p["b_ob2"]
    w_reward = p["w_reward"][:, :, 0, 0]  # [8, 16]
    b_reward = p["b_reward"]
    fc_r_w = p["fc_reward_w"]  # [4, 128]
    fc_r_b = p["fc_reward_b"]
    w_go = p["w_go"][:, :, 0, 0]
    b_go = p["b_go"]
    fc_g_w = p["fc_go_w"]
    fc_g_b = p["fc_go_b"]

    # conv1 as dense map: input features [68] = 64 ob one-hot (p*4+c) + 4 ac
    # one-hot; output features [512] = p_out*32 + o.
    W1 = np.zeros((68, 512), np.float32)
    for o in range(32):
        for i in range(H):
            for j in range(W):
                p_out = i * W + j
                col = p_out * 32 + o
                for di in (-1, 0, 1):
                    for dj in (-1, 0, 1):
                        ii, jj = i + di, j + dj
                        if 0 <= ii < H and 0 <= jj < W:
                            p_in = ii * W + jj
                            for c in range(N_PIECES):
                                W1[p_in * 4 + c, col] += w_start1[o, c, di + 1, dj + 1]
                            for a in range(N_ACTIONS):
                                W1[64 + a, col] += w_start1[o, 4 + a, di + 1, dj + 1]
    b1vec = np.tile(b_start1, 4).astype(np.float32)  # [128] partition q -> b[q%32]

    # start2: block-diag over 4 pixels per tile; same lhsT for all 4 matmuls
    W2 = np.zeros((128, 64), np.float32)
    for pl in range(4):
        W2[pl * 32 : pl * 32 + 32, pl * 16 : pl * 16 + 16] = w_start2.T
    b2vec = np.tile(b_start2, 8).astype(np.float32)

    # ob1: block-diag over 8 pixels
    Wob1 = np.zeros((128, 128), np.float32)
    for pl in range(8):
        Wob1[pl * 16 : pl * 16 + 16, pl * 16 : pl * 16 + 16] = w_ob1.T
    bob1vec = np.tile(b_ob1, 8).astype(np.float32)

    # ob2: out8 partitions laid out [64 loc | 64 scale], f = p*4 + c within
    # each half. Part a contracts nol (in-ch 0..15), split into loc/scale
    # lhsTs of M=32 (per nol tile u the outputs land at [u*32:u*32+32] and
    # [64+u*32 : 64+u*32+32]). Part b contracts the ob one-hot rows of xT.
    Wob2aL = np.zeros((128, 32), np.float32)
    Wob2aS = np.zeros((128, 32), np.float32)
    for pl in range(8):
        Wob2aL[pl * 16 : pl * 16 + 16, pl * 4 : pl * 4 + 4] = w_ob2[:4, :16].T
        Wob2aS[pl * 16 : pl * 16 + 16, pl * 4 : pl * 4 + 4] = w_ob2[4:, :16].T
    Wob2b = np.zeros((64, 128), np.float32)
    for pp in range(16):
        Wob2b[pp * 4 : pp * 4 + 4, pp * 4 : pp * 4 + 4] = w_ob2[:4, 16:].T
        Wob2b[pp * 4 : pp * 4 + 4, 64 + pp * 4 : 64 + pp * 4 + 4] = w_ob2[4:, 16:].T
    bob2vec = np.concatenate(
        [np.tile(b_ob2[:4], 16), np.tile(b_ob2[4:], 16)]).astype(np.float32)

    # reward+go convs packed: [128 in] -> [64 reward | 64 go]
    Wrg = np.zeros((128, 128), np.float32)
    for pl in range(8):
        Wrg[pl * 16 : pl * 16 + 16, pl * 8 : pl * 8 + 8] = w_reward.T
        Wrg[pl * 16 : pl * 16 + 16, 64 + pl * 8 : 64 + pl * 8 + 8] = w_go.T
    brgvec = np.concatenate([np.tile(b_reward, 8), np.tile(b_go, 8)]).astype(np.float32)

    # fc heads: contraction over rg tile u partitions -> 8 outputs
    Wfc = np.zeros((128, 16), np.float32)  # cols [u*8 : u*8+8]
    for u in range(2):
        for pl in range(8):
            pix = 8 * u + pl
            for o in range(8):
                k = o * 16 + pix
                for j in range(4):
                    Wfc[pl * 8 + o, u * 8 + j] = fc_r_w[j, k]
                    Wfc[64 + pl * 8 + o, u * 8 + 4 + j] = fc_g_w[j, k]
    fcvec = np.zeros(128, np.float32)
    fcvec[:4] = fc_r_b
    fcvec[4:8] = fc_g_b

    # bias matrix [128, 6]: cols = b1, b2, bob1, bob2, brg, fc
    biases = np.stack([b1vec, b2vec, bob1vec, bob2vec, brgvec, fcvec], 1)
    return dict(W1=W1, W2=W2, Wob1=Wob1, Wob2aL=Wob2aL, Wob2aS=Wob2aS,
                Wob2b=Wob2b, Wrg=Wrg, Wfc=Wfc, biases=biases)


# ---------------------------------------------------------------- device IR
def _build_program(ob_words_per_px: int, bc: int = BC):
    """Emit the Bass program. ob_words_per_px: 1 for int32 input, 2 for int64
    (we view the int64 buffer as int32 pairs and read the low words)."""
    nc = bacc.Bacc(None, target_bir_lowering=False, debug=False)
    mmdt = F32 if MM_DTYPE == "float32" else (BF16 if MM_DTYPE == "bfloat16" else F16)
    nt = bc // TILE_N

    obw = ob_words_per_px
    ob_d = nc.dram_tensor("ob", [bc, 16 * obw], I32, kind="ExternalInput")
    ac_d = nc.dram_tensor("ac", [bc * obw], I32, kind="ExternalInput")

    w1_d = nc.dram_tensor("W1", [68, 512], mmdt, kind="ExternalInput")
    w2_d = nc.dram_tensor("W2", [128, 64], mmdt, kind="ExternalInput")
    wob1_d = nc.dram_tensor("Wob1", [128, 128], mmdt, kind="ExternalInput")
    wob2al_d = nc.dram_tensor("Wob2aL", [128, 32], mmdt, kind="ExternalInput")
    wob2as_d = nc.dram_tensor("Wob2aS", [128, 32], mmdt, kind="ExternalInput")
    wob2b_d = nc.dram_tensor("Wob2b", [64, 128], mmdt, kind="ExternalInput")
    wrg_d = nc.dram_tensor("Wrg", [128, 128], mmdt, kind="ExternalInput")
    wfc_d = nc.dram_tensor("Wfc", [128, 16], mmdt, kind="ExternalInput")
    bias_d = nc.dram_tensor("biases", [128, 6], F32, kind="ExternalInput")
    iota_d = nc.dram_tensor("iota4", [128, 4], I32, kind="ExternalInput")

    loc_d = nc.dram_tensor("loc", [64, bc], F32, kind="ExternalOutput")
    scale_d = nc.dram_tensor("scale", [64, bc], F32, kind="ExternalOutput")
    rg_d = nc.dram_tensor("rg", [8, bc], F32, kind="ExternalOutput")

    with tile.TileContext(nc) as tc:
        with (
            tc.tile_pool(name="const", bufs=1) as cpool,
            tc.tile_pool(name="inp", bufs=3) as ipool,
            tc.tile_pool(name="oh", bufs=3) as ohpool,
            tc.tile_pool(name="xts", bufs=6) as xtpool,
            tc.tile_pool(name="act", bufs=3) as apool,
            tc.tile_pool(name="outs", bufs=3) as opool,
            tc.tile_pool(name="ps", bufs=4, space="PSUM") as pspool,
        ):
            # ---- constants
            w1_s = cpool.tile([68, 512], mmdt, tag="w1")
            w2_s = cpool.tile([128, 64], mmdt, tag="w2")
            wob1_s = cpool.tile([128, 128], mmdt, tag="wob1")
            wob2al_s = cpool.tile([128, 32], mmdt, tag="wob2al")
            wob2as_s = cpool.tile([128, 32], mmdt, tag="wob2as")
            wob2b_s = cpool.tile([64, 128], mmdt, tag="wob2b")
            wrg_s = cpool.tile([128, 128], mmdt, tag="wrg")
            wfc_s = cpool.tile([128, 16], mmdt, tag="wfc")
            bias_s = cpool.tile([128, 6], F32, tag="bias")
            iota_s = cpool.tile([128, 4], I32, tag="iota")
            for dst, src in [(iota_s, iota_d), (bias_s, bias_d),
                             (w1_s, w1_d), (w2_s, w2_d), (wob1_s, wob1_d),
                             (wrg_s, wrg_d), (wob2al_s, wob2al_d),
                             (wob2as_s, wob2as_d), (wob2b_s, wob2b_d),
                             (wfc_s, wfc_d)]:
                nc.sync.dma_start(dst[:], src[:])

            def bias_ap(col):
                return bias_s[:, col : col + 1]

            RELU = mybir.ActivationFunctionType.Relu
            ADD = mybir.AluOpType.add
            MAX = mybir.AluOpType.max
            EQ = mybir.AluOpType.is_equal

            def emit_A(it):
                n0 = it * TILE_N
                st = {}
                obn = ipool.tile([128, 64], I32, tag="obn", name=f"obn{it}")
                src = bass.AP(ob_d, n0 * 16 * obw,
                              [[16 * obw, 128], [128 * 16 * obw, 4], [obw, 16]])
                nc.gpsimd.dma_start(bass.AP(obn[:].tensor, obn[:].offset,
                                            [[64, 128], [16, 4], [1, 16]]), src)
                acn = ipool.tile([128, 4], I32, tag="acn", name=f"acn{it}")
                asrc = bass.AP(ac_d, n0 * obw, [[obw, 128], [128 * obw, 4]])
                nc.gpsimd.dma_start(acn[:], asrc)

                xoh = ohpool.tile([128, 512], mmdt, tag="xoh", name=f"xoh{it}")
                th = xoh[:].tensor
                xo = xoh[:].offset
                nc.vector.tensor_tensor(
                    bass.AP(th, xo, [[512, 128], [128, 4], [4, 16], [1, 4]]),
                    bass.AP(obn[:].tensor, obn[:].offset,
                            [[64, 128], [16, 4], [1, 16], [0, 4]]),
                    bass.AP(iota_s[:].tensor, iota_s[:].offset,
                            [[4, 128], [0, 4], [0, 16], [1, 4]]),
                    EQ,
                )
                nc.vector.tensor_tensor(
                    bass.AP(th, xo + 64, [[512, 128], [128, 4], [1, 4]]),
                    bass.AP(acn[:].tensor, acn[:].offset,
                            [[4, 128], [1, 4], [0, 4]]),
                    bass.AP(iota_s[:].tensor, iota_s[:].offset,
                            [[4, 128], [0, 4], [1, 4]]),
                    EQ,
                )
                if it < 3:
                    # keep the never-read transposed tail finite for the sim
                    nc.gpsimd.memset(
                        bass.AP(th, xo + 68, [[512, 128], [128, 4], [1, 60]]),
                        0.0)
                xt = xtpool.tile([128, 512], mmdt, tag="xt", name=f"xt{it}")
                nc.sync.dma_start_transpose(
                    bass.AP(xt[:].tensor, xt[:].offset,
                            [[512, 128], [128, 4], [1, 128]]),
                    xoh[:])
                st["xt"] = xt
                return st

            def emit_A1(it, st):
                xt = st["xt"]
                x1ps = [pspool.tile([128, 1024], F32, tag="ps",
                                    name=f"x1ps{it}_{_i}") for _i in range(2)]
                for m in range(4):
                    nc.tensor.matmul(
                        x1ps[m // 2][:, (m % 2) * 512 : (m % 2) * 512 + 512],
                        w1_s[:, m * 128 : (m + 1) * 128],
                        xt[0:68, :], start=True, stop=True)
                x1 = [apool.tile([128, 1024], mmdt, tag="x1",
                                 name=f"x1_{it}_{_i}") for _i in range(2)]
                nc.scalar.activation(x1[0][:], x1ps[0][:], RELU, bias=bias_ap(0))
                nc.vector.tensor_scalar(x1[1][:], x1ps[1][:], bias_ap(0), 0.0,
                                        ADD, MAX)
                st["x1"] = x1

            def emit_B1(it, st):
                x2ps = pspool.tile([128, 1024], F32, tag="ps", name=f"x2ps{it}")
                x1 = st["x1"]
                for u in range(2):
                    for h in range(2):
                        nc.tensor.matmul(
                            x2ps[h * 64 : h * 64 + 64, u * 512 : u * 512 + 512],
                            w2_s[:], x1[u][:, h * 512 : h * 512 + 512],
                            start=True, stop=True, skip_group_check=True)
                x2 = apool.tile([128, 1024], mmdt, tag="x2", name=f"x2_{it}")
                nc.scalar.activation(x2[:], x2ps[:], RELU, bias=bias_ap(1))
                st["x2"] = x2

            def emit_B2(it, st):
                x2 = st["x2"]
                nolps = pspool.tile([128, 1024], F32, tag="ps", name=f"nolps{it}")
                for u in range(2):
                    nc.tensor.matmul(nolps[:, u * 512 : u * 512 + 512], wob1_s[:],
                                     x2[:, u * 512 : u * 512 + 512],
                                     start=True, stop=True)
                nol = apool.tile([128, 1024], mmdt, tag="nol", name=f"nol{it}")
                nc.scalar.activation(nol[:], nolps[:], RELU, bias=bias_ap(2))
                st["nol"] = nol

                rgps = pspool.tile([128, 1024], F32, tag="ps", name=f"rgps{it}")
                for u in range(2):
                    nc.tensor.matmul(rgps[:, u * 512 : u * 512 + 512], wrg_s[:],
                                     x2[:, u * 512 : u * 512 + 512],
                                     start=True, stop=True)
                rg = apool.tile([128, 1024], mmdt, tag="rg", name=f"rg{it}")
                nc.scalar.activation(rg[:, 0:512], rgps[:, 0:512], RELU,
                                     bias=bias_ap(4))
                nc.vector.tensor_scalar(rg[:, 512:1024], rgps[:, 512:1024],
                                        bias_ap(4), 0.0, ADD, MAX)
                st["rg"] = rg

            def emit_C(it, st):
                n0 = it * TILE_N
                xt = st["xt"]
                nol = st["nol"]
                rg = st["rg"]
                late = pspool.tile([128, 1024], F32, tag="ps", name=f"late{it}")
                o8ps = late[:, 0:512]
                nc.tensor.matmul(late[0:32, 0:512], wob2al_s[:],
                                 nol[:, 0:512], start=True, stop=False,
                                 skip_group_check=True)
                nc.tensor.matmul(late[32:64, 0:512], wob2al_s[:],
                                 nol[:, 512:1024], start=True, stop=False,
                                 skip_group_check=True)
                nc.tensor.matmul(late[64:96, 0:512], wob2as_s[:],
                                 nol[:, 0:512], start=True, stop=False,
                                 skip_group_check=True)
                nc.tensor.matmul(late[96:128, 0:512], wob2as_s[:],
                                 nol[:, 512:1024], start=True, stop=False,
                                 tile_position=(0, 96), skip_group_check=True)
                nc.tensor.matmul(o8ps, wob2b_s[:], xt[0:64, :],
                                 start=False, stop=True, skip_group_check=True)
                nc.tensor.matmul(late[0:8, 512:1024], wfc_s[:, 0:8],
                                 rg[:, 0:512], start=True, stop=False,
                                 skip_group_check=True)
                nc.tensor.matmul(late[0:8, 512:1024], wfc_s[:, 8:16],
                                 rg[:, 512:1024], start=False, stop=True,
                                 skip_group_check=True)

                o8 = opool.tile([128, 512], F32, tag="o8", name=f"o8_{it}")
                nc.vector.tensor_scalar(o8[:], o8ps, bias_ap(3), None, ADD)
                fcsb = opool.tile([8, 512], F32, tag="fcsb", name=f"fcsb{it}")
                nc.vector.tensor_scalar(fcsb[:], late[0:8, 512:1024],
                                        bias_s[0:8, 5:6], None, ADD)

                # feature-major stores: loc/scale [64, bc], rg [8, bc]
                nc.sync.dma_start(
                    bass.AP(loc_d, n0, [[bc, 64], [1, 512]]), o8[0:64, :])
                nc.sync.dma_start(
                    bass.AP(scale_d, n0, [[bc, 64], [1, 512]]), o8[64:128, :])
                nc.sync.dma_start(
                    bass.AP(rg_d, n0, [[bc, 8], [1, 512]]), fcsb[:])

            states = {}
            for i in range(nt + 4):
                if i < nt:
                    states[i] = emit_A(i)
                if 0 <= i - 1 < nt:
                    emit_A1(i - 1, states[i - 1])
                if 0 <= i - 2 < nt:
                    emit_B1(i - 2, states[i - 2])
                if 0 <= i - 3 < nt:
                    emit_B2(i - 3, states[i - 3])
                if 0 <= i - 4 < nt:
                    emit_C(i - 4, states[i - 4])
                    del states[i - 4]

    nc.compile()
    return nc


_PROG_CACHE = {}


def _get_program(obw):
    if obw not in _PROG_CACHE:
        _PROG_CACHE[obw] = _build_program(obw)
    return _PROG_CACHE[obw]


def kernel(ob, ac, w_start1, b_start1, w_start2, b_start2, w_ob1, b_ob1,
           w_ob2, b_ob2, w_reward, b_reward, fc_reward_w, fc_reward_b,
           w_go, b_go, fc_go_w, fc_go_b):
    ob = np.asarray(ob)
    ac = np.asarray(ac)
    B = ob.shape[0]
    assert B == B_TOTAL, f"kernel hardcoded for B={B_TOTAL}, got {B}"

    params = dict(w_start1=w_start1, b_start1=b_start1, w_start2=w_start2,
                  b_start2=b_start2, w_ob1=w_ob1, b_ob1=b_ob1, w_ob2=w_ob2,
                  b_ob2=b_ob2, w_reward=w_reward, b_reward=b_reward,
                  fc_reward_w=fc_reward_w, fc_reward_b=fc_reward_b,
                  w_go=w_go, b_go=b_go, fc_go_w=fc_go_w, fc_go_b=fc_go_b)
    params = {k: np.asarray(v, np.float32) for k, v in params.items()}
    mats = _build_weight_mats(params)

    # index tensors: values are 0..3, so int32 is lossless for any int dtype
    obw = 1
    ob_v = np.ascontiguousarray(ob.reshape(B, 16)).astype(np.int32, copy=False)
    ac_v = np.ascontiguousarray(ac).astype(np.int32, copy=False)

    nc = _get_program(obw)

    mmnp = np.float32 if MM_DTYPE == "float32" else None
    def cast_mm(x):
        if MM_DTYPE == "float32":
            return np.asarray(x, np.float32)
        import ml_dtypes
        return np.asarray(x, ml_dtypes.bfloat16)

    consts = {
        "W1": cast_mm(mats["W1"]), "W2": cast_mm(mats["W2"]),
        "Wob1": cast_mm(mats["Wob1"]), "Wob2aL": cast_mm(mats["Wob2aL"]),
        "Wob2aS": cast_mm(mats["Wob2aS"]),
        "Wob2b": cast_mm(mats["Wob2b"]), "Wrg": cast_mm(mats["Wrg"]),
        "Wfc": cast_mm(mats["Wfc"]),
        "biases": mats["biases"],
        "iota4": np.tile(np.arange(4, dtype=np.int32), (128, 1)),
    }

    in_maps = []
    for c in range(NCORES):
        sl = slice(c * BC, (c + 1) * BC)
        in_maps.append({"ob": np.ascontiguousarray(ob_v[sl]),
                        "ac": np.ascontiguousarray(ac_v[c * BC * obw:(c + 1) * BC * obw]),
                        **consts})

    import os
    trace = bool(int(os.environ.get("KERNEL_TRACE", "0")))
    res = run_bass_kernel_spmd(nc, in_maps, core_ids=list(range(NCORES)),
                               trace=trace)
    global _LAST_RESULT
    _LAST_RESULT = res

    # device emits feature-major [64, bc] / [8, bc]; transpose on host
    loc = np.concatenate([res.results[c]["loc"] for c in range(NCORES)], 1)
    scale = np.concatenate([res.results[c]["scale"] for c in range(NCORES)], 1)
    rg = np.concatenate([res.results[c]["rg"] for c in range(NCORES)], 1)

    next_ob_loc = np.ascontiguousarray(loc.T).reshape(B, H, W, N_PIECES)
    next_ob_scale = np.ascontiguousarray(scale.T).reshape(B, H, W, N_PIECES)
    rg = rg.T
    reward_loc = np.ascontiguousarray(rg[:, 0:2])
    reward_scale = np.ascontiguousarray(rg[:, 2:4])
    go_loc = np.ascontiguousarray(rg[:, 4:6])
    go_scale = np.ascontiguousarray(rg[:, 6:8])
    return (next_ob_loc, next_ob_scale, reward_loc, reward_scale,
            go_loc, go_scale)


# revision 37
# speedup vs baseline: 1.2884x; 1.2884x over previous
"""Trainium2 Bass kernel for the DreamWorld dense-CNN model.

Contract: kernel(**inputs) takes the FULL unsharded numpy inputs (as produced
by the reference setup_inputs) and returns the full outputs. Internally the
batch is sharded across 8 NeuronCores (pure data parallel); the small weights
are replicated. All shapes/strategy are hardcoded.
"""

import numpy as np

import concourse.bacc as bacc
import concourse.bass as bass
import concourse.mybir as mybir
import concourse.tile as tile
from concourse.bass_utils import run_bass_kernel_spmd

N_PIECES = 4
N_ACTIONS = 4
H = W = 4
NPIX = H * W
B_TOTAL = 131072
NCORES = 8
BC = B_TOTAL // NCORES          # per-core batch
TILE_N = 512                    # samples per device tile
NT = BC // TILE_N               # tiles per core
NCHUNK = TILE_N // 128          # 128-sample chunks per tile

F32 = mybir.dt.float32
BF16 = mybir.dt.bfloat16
F16 = mybir.dt.float16
I32 = mybir.dt.int32

# matmul/activation compute dtype for the hidden layers ("float32",
# "bfloat16"); outputs always produced/stored in f32.
MM_DTYPE = "bfloat16"


# ---------------------------------------------------------------- host math
def _build_weight_mats(p):
    """Transform reference conv/fc weights into the dense feature-major
    matmul operands used on device. All f32 numpy."""
    w_start1 = p["w_start1"]  # [32, 8, 3, 3]
    b_start1 = p["b_start1"]
    w_start2 = p["w_start2"][:, :, 0, 0]  # [16, 32]
    b_start2 = p["b_start2"]
    w_ob1 = p["w_ob1"][:, :, 0, 0]  # [16, 16]
    b_ob1 = p["b_ob1"]
    w_ob2 = p["w_ob2"][:, :, 0, 0]  # [8, 20]
    b_ob2 = p["b_ob2"]
    w_reward = p["w_reward"][:, :, 0, 0]  # [8, 16]
    b_reward = p["b_reward"]
    fc_r_w = p["fc_reward_w"]  # [4, 128]
    fc_r_b = p["fc_reward_b"]
    w_go = p["w_go"][:, :, 0, 0]
    b_go = p["b_go"]
    fc_g_w = p["fc_go_w"]
    fc_g_b = p["fc_go_b"]

    # conv1 as dense map: input features [68] = 64 ob one-hot (p*4+c) + 4 ac
    # one-hot; output features [512] = p_out*32 + o.
    W1 = np.zeros((68, 512), np.float32)
    for o in range(32):
        for i in range(H):
            for j in range(W):
                p_out = i * W + j
                col = p_out * 32 + o
                for di in (-1, 0, 1):
                    for dj in (-1, 0, 1):
                        ii, jj = i + di, j + dj
                        if 0 <= ii < H and 0 <= jj < W:
                            p_in = ii * W + jj
                            for c in range(N_PIECES):
                                W1[p_in * 4 + c, col] += w_start1[o, c, di + 1, dj + 1]
                            for a in range(N_ACTIONS):
                                W1[64 + a, col] += w_start1[o, 4 + a, di + 1, dj + 1]
    b1vec = np.tile(b_start1, 4).astype(np.float32)  # [128] partition q -> b[q%32]

    # start2: block-diag over 4 pixels per tile; same lhsT for all 4 matmuls
    W2 = np.zeros((128, 64), np.float32)
    for pl in range(4):
        W2[pl * 32 : pl * 32 + 32, pl * 16 : pl * 16 + 16] = w_start2.T
    b2vec = np.tile(b_start2, 8).astype(np.float32)

    # ob1: block-diag over 8 pixels
    Wob1 = np.zeros((128, 128), np.float32)
    for pl in range(8):
        Wob1[pl * 16 : pl * 16 + 16, pl * 16 : pl * 16 + 16] = w_ob1.T
    bob1vec = np.tile(b_ob1, 8).astype(np.float32)

    # ob2: out8 partitions laid out [64 loc | 64 scale], f = p*4 + c within
    # each half. Part a contracts nol (in-ch 0..15), split into loc/scale
    # lhsTs of M=32 (per nol tile u the outputs land at [u*32:u*32+32] and
    # [64+u*32 : 64+u*32+32]). Part b contracts the ob one-hot rows of xT.
    Wob2aL = np.zeros((128, 32), np.float32)
    Wob2aS = np.zeros((128, 32), np.float32)
    for pl in range(8):
        Wob2aL[pl * 16 : pl * 16 + 16, pl * 4 : pl * 4 + 4] = w_ob2[:4, :16].T
        Wob2aS[pl * 16 : pl * 16 + 16, pl * 4 : pl * 4 + 4] = w_ob2[4:, :16].T
    Wob2b = np.zeros((64, 128), np.float32)
    for pp in range(16):
        Wob2b[pp * 4 : pp * 4 + 4, pp * 4 : pp * 4 + 4] = w_ob2[:4, 16:].T
        Wob2b[pp * 4 : pp * 4 + 4, 64 + pp * 4 : 64 + pp * 4 + 4] = w_ob2[4:, 16:].T
    bob2vec = np.concatenate(
        [np.tile(b_ob2[:4], 16), np.tile(b_ob2[4:], 16)]).astype(np.float32)

    # reward+go convs packed: [128 in] -> [64 reward | 64 go]
    Wrg = np.zeros((128, 128), np.float32)
    for pl in range(8):
        Wrg[pl * 16 : pl * 16 + 16, pl * 8 : pl * 8 + 8] = w_reward.T
        Wrg[pl * 16 : pl * 16 + 16, 64 + pl * 8 : 64 + pl * 8 + 8] = w_go.T
    brgvec = np.concatenate([np.tile(b_reward, 8), np.tile(b_go, 8)]).astype(np.float32)

    # fc heads: contraction over rg tile u partitions -> 8 outputs
    Wfc = np.zeros((128, 16), np.float32)  # cols [u*8 : u*8+8]
    for u in range(2):
        for pl in range(8):
            pix = 8 * u + pl
            for o in range(8):
                k = o * 16 + pix
                for j in range(4):
                    Wfc[pl * 8 + o, u * 8 + j] = fc_r_w[j, k]
                    Wfc[64 + pl * 8 + o, u * 8 + 4 + j] = fc_g_w[j, k]
    fcvec = np.zeros(128, np.float32)
    fcvec[:4] = fc_r_b
    fcvec[4:8] = fc_g_b

    # bias matrix [128, 6]: cols = b1, b2, bob1, bob2, brg, fc
    biases = np.stack([b1vec, b2vec, bob1vec, bob2vec, brgvec, fcvec], 1)
    return dict(W1=W1, W2=W2, Wob1=Wob1, Wob2aL=Wob2aL, Wob2aS=Wob2aS,
                Wob2b=Wob2b, Wrg=Wrg, Wfc=Wfc, biases=biases)


# ---------------------------------------------------------------- device IR
def _build_program(ob_words_per_px: int, bc: int = BC):
    """Emit the Bass program. ob_words_per_px: 1 for int32 input, 2 for int64
    (we view the int64 buffer as int32 pairs and read the low words)."""
    nc = bacc.Bacc(None, target_bir_lowering=False, debug=False)
    mmdt = F32 if MM_DTYPE == "float32" else (BF16 if MM_DTYPE == "bfloat16" else F16)
    nt = bc // TILE_N

    obw = ob_words_per_px
    ob_d = nc.dram_tensor("ob", [bc, 16 * obw], I32, kind="ExternalInput")
    ac_d = nc.dram_tensor("ac", [bc * obw], I32, kind="ExternalInput")

    w1_d = nc.dram_tensor("W1", [68, 512], mmdt, kind="ExternalInput")
    w2_d = nc.dram_tensor("W2", [128, 64], mmdt, kind="ExternalInput")
    wob1_d = nc.dram_tensor("Wob1", [128, 128], mmdt, kind="ExternalInput")
    wob2al_d = nc.dram_tensor("Wob2aL", [128, 32], mmdt, kind="ExternalInput")
    wob2as_d = nc.dram_tensor("Wob2aS", [128, 32], mmdt, kind="ExternalInput")
    wob2b_d = nc.dram_tensor("Wob2b", [64, 128], mmdt, kind="ExternalInput")
    wrg_d = nc.dram_tensor("Wrg", [128, 128], mmdt, kind="ExternalInput")
    wfc_d = nc.dram_tensor("Wfc", [128, 16], mmdt, kind="ExternalInput")
    bias_d = nc.dram_tensor("biases", [128, 6], F32, kind="ExternalInput")
    iota_d = nc.dram_tensor("iota4", [128, 4], I32, kind="ExternalInput")

    loc_d = nc.dram_tensor("loc", [64, bc], F32, kind="ExternalOutput")
    scale_d = nc.dram_tensor("scale", [64, bc], F32, kind="ExternalOutput")
    rg_d = nc.dram_tensor("rg", [8, bc], F32, kind="ExternalOutput")

    with tile.TileContext(nc) as tc:
        with (
            tc.tile_pool(name="const", bufs=1) as cpool,
            tc.tile_pool(name="inp", bufs=3) as ipool,
            tc.tile_pool(name="oh", bufs=3) as ohpool,
            tc.tile_pool(name="xts", bufs=6) as xtpool,
            tc.tile_pool(name="act", bufs=3) as apool,
            tc.tile_pool(name="outs", bufs=3) as opool,
            tc.tile_pool(name="ps", bufs=4, space="PSUM") as pspool,
        ):
            # ---- constants
            w1_s = cpool.tile([68, 512], mmdt, tag="w1")
            w2_s = cpool.tile([128, 64], mmdt, tag="w2")
            wob1_s = cpool.tile([128, 128], mmdt, tag="wob1")
            wob2al_s = cpool.tile([128, 32], mmdt, tag="wob2al")
            wob2as_s = cpool.tile([128, 32], mmdt, tag="wob2as")
            wob2b_s = cpool.tile([64, 128], mmdt, tag="wob2b")
            wrg_s = cpool.tile([128, 128], mmdt, tag="wrg")
            wfc_s = cpool.tile([128, 16], mmdt, tag="wfc")
            bias_s = cpool.tile([128, 6], F32, tag="bias")
            iota_s = cpool.tile([128, 4], I32, tag="iota")
            for dst, src in [(iota_s, iota_d), (bias_s, bias_d),
                             (w1_s, w1_d), (w2_s, w2_d), (wob1_s, wob1_d),
                             (wrg_s, wrg_d), (wob2al_s, wob2al_d),
                             (wob2as_s, wob2as_d), (wob2b_s, wob2b_d),
                             (wfc_s, wfc_d)]:
                nc.sync.dma_start(dst[:], src[:])

            def bias_ap(col):
                return bias_s[:, col : col + 1]

            RELU = mybir.ActivationFunctionType.Relu
            ADD = mybir.AluOpType.add
            MAX = mybir.AluOpType.max
            EQ = mybir.AluOpType.is_equal

            def emit_A(it):
                n0 = it * TILE_N
                st = {}
                obn = ipool.tile([128, 64], I32, tag="obn", name=f"obn{it}")
                src = bass.AP(ob_d, n0 * 16 * obw,
                              [[16 * obw, 128], [128 * 16 * obw, 4], [obw, 16]])
                nc.gpsimd.dma_start(bass.AP(obn[:].tensor, obn[:].offset,
                                            [[64, 128], [16, 4], [1, 16]]), src)
                acn = ipool.tile([128, 4], I32, tag="acn", name=f"acn{it}")
                asrc = bass.AP(ac_d, n0 * obw, [[obw, 128], [128 * obw, 4]])
                nc.gpsimd.dma_start(acn[:], asrc)

                xoh = ohpool.tile([128, 512], mmdt, tag="xoh", name=f"xoh{it}")
                th = xoh[:].tensor
                xo = xoh[:].offset
                nc.vector.tensor_tensor(
                    bass.AP(th, xo, [[512, 128], [128, 4], [4, 16], [1, 4]]),
                    bass.AP(obn[:].tensor, obn[:].offset,
                            [[64, 128], [16, 4], [1, 16], [0, 4]]),
                    bass.AP(iota_s[:].tensor, iota_s[:].offset,
                            [[4, 128], [0, 4], [0, 16], [1, 4]]),
                    EQ,
                )
                nc.vector.tensor_tensor(
                    bass.AP(th, xo + 64, [[512, 128], [128, 4], [1, 4]]),
                    bass.AP(acn[:].tensor, acn[:].offset,
                            [[4, 128], [1, 4], [0, 4]]),
                    bass.AP(iota_s[:].tensor, iota_s[:].offset,
                            [[4, 128], [0, 4], [1, 4]]),
                    EQ,
                )
                if it < 3:
                    # keep the never-read transposed tail finite for the sim
                    nc.gpsimd.memset(
                        bass.AP(th, xo + 68, [[512, 128], [128, 4], [1, 60]]),
                        0.0)
                xt = xtpool.tile([128, 512], mmdt, tag="xt", name=f"xt{it}")
                nc.sync.dma_start_transpose(
                    bass.AP(xt[:].tensor, xt[:].offset,
                            [[512, 128], [128, 4], [1, 128]]),
                    xoh[:])
                st["xt"] = xt
                return st

            def emit_A1(it, st):
                xt = st["xt"]
                x1ps = [pspool.tile([128, 1024], F32, tag="ps",
                                    name=f"x1ps{it}_{_i}") for _i in range(2)]
                for m in range(4):
                    nc.tensor.matmul(
                        x1ps[m // 2][:, (m % 2) * 512 : (m % 2) * 512 + 512],
                        w1_s[:, m * 128 : (m + 1) * 128],
                        xt[0:68, :], start=True, stop=True)
                x1 = [apool.tile([128, 1024], mmdt, tag="x1",
                                 name=f"x1_{it}_{_i}") for _i in range(2)]
                nc.scalar.activation(x1[0][:], x1ps[0][:], RELU, bias=bias_ap(0))
                nc.vector.tensor_scalar(x1[1][:], x1ps[1][:], bias_ap(0), 0.0,
                                        ADD, MAX)
                st["x1"] = x1

            def emit_B1(it, st):
                x2ps = pspool.tile([128, 1024], F32, tag="ps", name=f"x2ps{it}")
                x1 = st["x1"]
                for u in range(2):
                    for h in range(2):
                        nc.tensor.matmul(
                            x2ps[h * 64 : h * 64 + 64, u * 512 : u * 512 + 512],
                            w2_s[:], x1[u][:, h * 512 : h * 512 + 512],
                            start=True, stop=True, skip_group_check=True)
                x2 = apool.tile([128, 1024], mmdt, tag="x2", name=f"x2_{it}")
                nc.scalar.activation(x2[:], x2ps[:], RELU, bias=bias_ap(1))
                st["x2"] = x2

            def emit_B2(it, st):
                x2 = st["x2"]
                nolps = pspool.tile([128, 1024], F32, tag="ps", name=f"nolps{it}")
                for u in range(2):
                    nc.tensor.matmul(nolps[:, u * 512 : u * 512 + 512], wob1_s[:],
                                     x2[:, u * 512 : u * 512 + 512],
                                     start=True, stop=True)
                nol = apool.tile([128, 1024], mmdt, tag="nol", name=f"nol{it}")
                nc.scalar.activation(nol[:], nolps[:], RELU, bias=bias_ap(2))
                st["nol"] = nol

                rgps = pspool.tile([128, 1024], F32, tag="ps", name=f"rgps{it}")
                for u in range(2):
                    nc.tensor.matmul(rgps[:, u * 512 : u * 512 + 512], wrg_s[:],
                                     x2[:, u * 512 : u * 512 + 512],
                                     start=True, stop=True)
                rg = apool.tile([128, 1024], mmdt, tag="rg", name=f"rg{it}")
                nc.vector.tensor_scalar(rg[:], rgps[:], bias_ap(4), 0.0, ADD, MAX)
                st["rg"] = rg

            def emit_C(it, st):
                n0 = it * TILE_N
                xt = st["xt"]
                nol = st["nol"]
                rg = st["rg"]
                late = pspool.tile([128, 1024], F32, tag="ps", name=f"late{it}")
                o8ps = late[:, 0:512]
                nc.tensor.matmul(late[0:32, 0:512], wob2al_s[:],
                                 nol[:, 0:512], start=True, stop=False,
                                 skip_group_check=True)
                nc.tensor.matmul(late[32:64, 0:512], wob2al_s[:],
                                 nol[:, 512:1024], start=True, stop=False,
                                 skip_group_check=True)
                nc.tensor.matmul(late[64:96, 0:512], wob2as_s[:],
                                 nol[:, 0:512], start=True, stop=False,
                                 skip_group_check=True)
                nc.tensor.matmul(late[96:128, 0:512], wob2as_s[:],
                                 nol[:, 512:1024], start=True, stop=False,
                                 tile_position=(0, 96), skip_group_check=True)
                nc.tensor.matmul(o8ps, wob2b_s[:], xt[0:64, :],
                                 start=False, stop=True, skip_group_check=True)
                nc.tensor.matmul(late[0:8, 512:1024], wfc_s[:, 0:8],
                                 rg[:, 0:512], start=True, stop=False,
                                 skip_group_check=True)
                nc.tensor.matmul(late[0:8, 512:1024], wfc_s[:, 8:16],
                                 rg[:, 512:1024], start=False, stop=True,
                                 skip_group_check=True)

                o8 = opool.tile([128, 512], F32, tag="o8", name=f"o8_{it}")
                nc.vector.tensor_scalar(o8[:], o8ps, bias_ap(3), None, ADD)
                fcsb = opool.tile([8, 512], F32, tag="fcsb", name=f"fcsb{it}")
                nc.vector.tensor_scalar(fcsb[:], late[0:8, 512:1024],
                                        bias_s[0:8, 5:6], None, ADD)

                # feature-major stores: loc/scale [64, bc], rg [8, bc]
                nc.sync.dma_start(
                    bass.AP(loc_d, n0, [[bc, 64], [1, 512]]), o8[0:64, :])
                nc.sync.dma_start(
                    bass.AP(scale_d, n0, [[bc, 64], [1, 512]]), o8[64:128, :])
                nc.sync.dma_start(
                    bass.AP(rg_d, n0, [[bc, 8], [1, 512]]), fcsb[:])

            states = {}
            for i in range(nt + 4):
                if i < nt:
                    states[i] = emit_A(i)
                if 0 <= i - 1 < nt:
                    emit_A1(i - 1, states[i - 1])
                if 0 <= i - 2 < nt:
                    emit_B1(i - 2, states[i - 2])
                if 0 <= i - 3 < nt:
                    emit_B2(i - 3, states[i - 3])
                if 0 <= i - 4 < nt:
                    emit_C(i - 4, states[i - 4])
                    del states[i - 4]

    nc.compile()
    return nc


_PROG_CACHE = {}


def _get_program(obw):
    if obw not in _PROG_CACHE:
        _PROG_CACHE[obw] = _build_program(obw)
    return _PROG_CACHE[obw]


def kernel(ob, ac, w_start1, b_start1, w_start2, b_start2, w_ob1, b_ob1,
           w_ob2, b_ob2, w_reward, b_reward, fc_reward_w, fc_reward_b,
           w_go, b_go, fc_go_w, fc_go_b):
    ob = np.asarray(ob)
    ac = np.asarray(ac)
    B = ob.shape[0]
    assert B == B_TOTAL, f"kernel hardcoded for B={B_TOTAL}, got {B}"

    params = dict(w_start1=w_start1, b_start1=b_start1, w_start2=w_start2,
                  b_start2=b_start2, w_ob1=w_ob1, b_ob1=b_ob1, w_ob2=w_ob2,
                  b_ob2=b_ob2, w_reward=w_reward, b_reward=b_reward,
                  fc_reward_w=fc_reward_w, fc_reward_b=fc_reward_b,
                  w_go=w_go, b_go=b_go, fc_go_w=fc_go_w, fc_go_b=fc_go_b)
    params = {k: np.asarray(v, np.float32) for k, v in params.items()}
    mats = _build_weight_mats(params)

    # index tensors: values are 0..3, so int32 is lossless for any int dtype
    obw = 1
    ob_v = np.ascontiguousarray(ob.reshape(B, 16)).astype(np.int32, copy=False)
    ac_v = np.ascontiguousarray(ac).astype(np.int32, copy=False)

    nc = _get_program(obw)

    mmnp = np.float32 if MM_DTYPE == "float32" else None
    def cast_mm(x):
        if MM_DTYPE == "float32":
            return np.asarray(x, np.float32)
        import ml_dtypes
        return np.asarray(x, ml_dtypes.bfloat16)

    consts = {
        "W1": cast_mm(mats["W1"]), "W2": cast_mm(mats["W2"]),
        "Wob1": cast_mm(mats["Wob1"]), "Wob2aL": cast_mm(mats["Wob2aL"]),
        "Wob2aS": cast_mm(mats["Wob2aS"]),
        "Wob2b": cast_mm(mats["Wob2b"]), "Wrg": cast_mm(mats["Wrg"]),
        "Wfc": cast_mm(mats["Wfc"]),
        "biases": mats["biases"],
        "iota4": np.tile(np.arange(4, dtype=np.int32), (128, 1)),
    }

    in_maps = []
    for c in range(NCORES):
        sl = slice(c * BC, (c + 1) * BC)
        in_maps.append({"ob": np.ascontiguousarray(ob_v[sl]),
                        "ac": np.ascontiguousarray(ac_v[c * BC * obw:(c + 1) * BC * obw]),
                        **consts})

    import os
    trace = bool(int(os.environ.get("KERNEL_TRACE", "0")))
    res = run_bass_kernel_spmd(nc, in_maps, core_ids=list(range(NCORES)),
                               trace=trace)
    global _LAST_RESULT
    _LAST_RESULT = res

    # device emits feature-major [64, bc] / [8, bc]; transpose on host
    loc = np.concatenate([res.results[c]["loc"] for c in range(NCORES)], 1)
    scale = np.concatenate([res.results[c]["scale"] for c in range(NCORES)], 1)
    rg = np.concatenate([res.results[c]["rg"] for c in range(NCORES)], 1)

    next_ob_loc = np.ascontiguousarray(loc.T).reshape(B, H, W, N_PIECES)
    next_ob_scale = np.ascontiguousarray(scale.T).reshape(B, H, W, N_PIECES)
    rg = rg.T
    reward_loc = np.ascontiguousarray(rg[:, 0:2])
    reward_scale = np.ascontiguousarray(rg[:, 2:4])
    go_loc = np.ascontiguousarray(rg[:, 4:6])
    go_scale = np.ascontiguousarray(rg[:, 6:8])
    return (next_ob_loc, next_ob_scale, reward_loc, reward_scale,
            go_loc, go_scale)


# revision 38
# speedup vs baseline: 1.2990x; 1.0082x over previous
"""Trainium2 Bass kernel for the DreamWorld dense-CNN model.

Contract: kernel(**inputs) takes the FULL unsharded numpy inputs (as produced
by the reference setup_inputs) and returns the full outputs. Internally the
batch is sharded across 8 NeuronCores (pure data parallel); the small weights
are replicated. All shapes/strategy are hardcoded.
"""

import numpy as np

import concourse.bacc as bacc
import concourse.bass as bass
import concourse.mybir as mybir
import concourse.tile as tile
from concourse.bass_utils import run_bass_kernel_spmd

N_PIECES = 4
N_ACTIONS = 4
H = W = 4
NPIX = H * W
B_TOTAL = 131072
NCORES = 8
BC = B_TOTAL // NCORES          # per-core batch
TILE_N = 512                    # samples per device tile
NT = BC // TILE_N               # tiles per core
NCHUNK = TILE_N // 128          # 128-sample chunks per tile

F32 = mybir.dt.float32
BF16 = mybir.dt.bfloat16
F16 = mybir.dt.float16
I32 = mybir.dt.int32

# matmul/activation compute dtype for the hidden layers ("float32",
# "bfloat16"); outputs always produced/stored in f32.
MM_DTYPE = "bfloat16"


# ---------------------------------------------------------------- host math
def _build_weight_mats(p):
    """Transform reference conv/fc weights into the dense feature-major
    matmul operands used on device. All f32 numpy."""
    w_start1 = p["w_start1"]  # [32, 8, 3, 3]
    b_start1 = p["b_start1"]
    w_start2 = p["w_start2"][:, :, 0, 0]  # [16, 32]
    b_start2 = p["b_start2"]
    w_ob1 = p["w_ob1"][:, :, 0, 0]  # [16, 16]
    b_ob1 = p["b_ob1"]
    w_ob2 = p["w_ob2"][:, :, 0, 0]  # [8, 20]
    b_ob2 = p["b_ob2"]
    w_reward = p["w_reward"][:, :, 0, 0]  # [8, 16]
    b_reward = p["b_reward"]
    fc_r_w = p["fc_reward_w"]  # [4, 128]
    fc_r_b = p["fc_reward_b"]
    w_go = p["w_go"][:, :, 0, 0]
    b_go = p["b_go"]
    fc_g_w = p["fc_go_w"]
    fc_g_b = p["fc_go_b"]

    # conv1 as dense map: input features [68] = 64 ob one-hot (p*4+c) + 4 ac
    # one-hot; output features [512] = p_out*32 + o.
    W1 = np.zeros((68, 512), np.float32)
    for o in range(32):
        for i in range(H):
            for j in range(W):
                p_out = i * W + j
                col = p_out * 32 + o
                for di in (-1, 0, 1):
                    for dj in (-1, 0, 1):
                        ii, jj = i + di, j + dj
                        if 0 <= ii < H and 0 <= jj < W:
                            p_in = ii * W + jj
                            for c in range(N_PIECES):
                                W1[p_in * 4 + c, col] += w_start1[o, c, di + 1, dj + 1]
                            for a in range(N_ACTIONS):
                                W1[64 + a, col] += w_start1[o, 4 + a, di + 1, dj + 1]
    b1vec = np.tile(b_start1, 4).astype(np.float32)  # [128] partition q -> b[q%32]

    # start2: block-diag over 4 pixels per tile; same lhsT for all 4 matmuls
    W2 = np.zeros((128, 64), np.float32)
    for pl in range(4):
        W2[pl * 32 : pl * 32 + 32, pl * 16 : pl * 16 + 16] = w_start2.T
    b2vec = np.tile(b_start2, 8).astype(np.float32)

    # ob1: block-diag over 8 pixels
    Wob1 = np.zeros((128, 128), np.float32)
    for pl in range(8):
        Wob1[pl * 16 : pl * 16 + 16, pl * 16 : pl * 16 + 16] = w_ob1.T
    bob1vec = np.tile(b_ob1, 8).astype(np.float32)

    # ob2: out8 partitions laid out [64 loc | 64 scale], f = p*4 + c within
    # each half. Part a contracts nol (in-ch 0..15), split into loc/scale
    # lhsTs of M=32 (per nol tile u the outputs land at [u*32:u*32+32] and
    # [64+u*32 : 64+u*32+32]). Part b contracts the ob one-hot rows of xT.
    Wob2aL = np.zeros((128, 32), np.float32)
    Wob2aS = np.zeros((128, 32), np.float32)
    for pl in range(8):
        Wob2aL[pl * 16 : pl * 16 + 16, pl * 4 : pl * 4 + 4] = w_ob2[:4, :16].T
        Wob2aS[pl * 16 : pl * 16 + 16, pl * 4 : pl * 4 + 4] = w_ob2[4:, :16].T
    Wob2b = np.zeros((64, 128), np.float32)
    for pp in range(16):
        Wob2b[pp * 4 : pp * 4 + 4, pp * 4 : pp * 4 + 4] = w_ob2[:4, 16:].T
        Wob2b[pp * 4 : pp * 4 + 4, 64 + pp * 4 : 64 + pp * 4 + 4] = w_ob2[4:, 16:].T
    bob2vec = np.concatenate(
        [np.tile(b_ob2[:4], 16), np.tile(b_ob2[4:], 16)]).astype(np.float32)

    # reward+go convs packed: [128 in] -> [64 reward | 64 go]
    Wrg = np.zeros((128, 128), np.float32)
    for pl in range(8):
        Wrg[pl * 16 : pl * 16 + 16, pl * 8 : pl * 8 + 8] = w_reward.T
        Wrg[pl * 16 : pl * 16 + 16, 64 + pl * 8 : 64 + pl * 8 + 8] = w_go.T
    brgvec = np.concatenate([np.tile(b_reward, 8), np.tile(b_go, 8)]).astype(np.float32)

    # fc heads: contraction over rg tile u partitions -> 8 outputs
    Wfc = np.zeros((128, 16), np.float32)  # cols [u*8 : u*8+8]
    for u in range(2):
        for pl in range(8):
            pix = 8 * u + pl
            for o in range(8):
                k = o * 16 + pix
                for j in range(4):
                    Wfc[pl * 8 + o, u * 8 + j] = fc_r_w[j, k]
                    Wfc[64 + pl * 8 + o, u * 8 + 4 + j] = fc_g_w[j, k]
    fcvec = np.zeros(128, np.float32)
    fcvec[:4] = fc_r_b
    fcvec[4:8] = fc_g_b

    # bias matrix [128, 6]: cols = b1, b2, bob1, bob2, brg, fc
    biases = np.stack([b1vec, b2vec, bob1vec, bob2vec, brgvec, fcvec], 1)
    return dict(W1=W1, W2=W2, Wob1=Wob1, Wob2aL=Wob2aL, Wob2aS=Wob2aS,
                Wob2b=Wob2b, Wrg=Wrg, Wfc=Wfc, biases=biases)


# ---------------------------------------------------------------- device IR
def _build_program(ob_words_per_px: int, bc: int = BC):
    """Emit the Bass program. ob_words_per_px: 1 for int32 input, 2 for int64
    (we view the int64 buffer as int32 pairs and read the low words)."""
    nc = bacc.Bacc(None, target_bir_lowering=False, debug=False)
    mmdt = F32 if MM_DTYPE == "float32" else (BF16 if MM_DTYPE == "bfloat16" else F16)
    nt = bc // TILE_N

    obw = ob_words_per_px
    ob_d = nc.dram_tensor("ob", [bc, 16 * obw], I32, kind="ExternalInput")
    ac_d = nc.dram_tensor("ac", [bc * obw], I32, kind="ExternalInput")

    w1_d = nc.dram_tensor("W1", [68, 512], mmdt, kind="ExternalInput")
    w2_d = nc.dram_tensor("W2", [128, 64], mmdt, kind="ExternalInput")
    wob1_d = nc.dram_tensor("Wob1", [128, 128], mmdt, kind="ExternalInput")
    wob2al_d = nc.dram_tensor("Wob2aL", [128, 32], mmdt, kind="ExternalInput")
    wob2as_d = nc.dram_tensor("Wob2aS", [128, 32], mmdt, kind="ExternalInput")
    wob2b_d = nc.dram_tensor("Wob2b", [64, 128], mmdt, kind="ExternalInput")
    wrg_d = nc.dram_tensor("Wrg", [128, 128], mmdt, kind="ExternalInput")
    wfc_d = nc.dram_tensor("Wfc", [128, 16], mmdt, kind="ExternalInput")
    bias_d = nc.dram_tensor("biases", [128, 6], F32, kind="ExternalInput")
    iota_d = nc.dram_tensor("iota4", [128, 4], I32, kind="ExternalInput")

    loc_d = nc.dram_tensor("loc", [64, bc], F32, kind="ExternalOutput")
    scale_d = nc.dram_tensor("scale", [64, bc], F32, kind="ExternalOutput")
    rg_d = nc.dram_tensor("rg", [8, bc], F32, kind="ExternalOutput")

    with tile.TileContext(nc) as tc:
        with (
            tc.tile_pool(name="const", bufs=1) as cpool,
            tc.tile_pool(name="inp", bufs=3) as ipool,
            tc.tile_pool(name="oh", bufs=3) as ohpool,
            tc.tile_pool(name="xts", bufs=6) as xtpool,
            tc.tile_pool(name="act", bufs=4) as apool,
            tc.tile_pool(name="outs", bufs=4) as opool,
            tc.tile_pool(name="ps", bufs=4, space="PSUM") as pspool,
        ):
            # ---- constants
            w1_s = cpool.tile([68, 512], mmdt, tag="w1")
            w2_s = cpool.tile([128, 64], mmdt, tag="w2")
            wob1_s = cpool.tile([128, 128], mmdt, tag="wob1")
            wob2al_s = cpool.tile([128, 32], mmdt, tag="wob2al")
            wob2as_s = cpool.tile([128, 32], mmdt, tag="wob2as")
            wob2b_s = cpool.tile([64, 128], mmdt, tag="wob2b")
            wrg_s = cpool.tile([128, 128], mmdt, tag="wrg")
            wfc_s = cpool.tile([128, 16], mmdt, tag="wfc")
            bias_s = cpool.tile([128, 6], F32, tag="bias")
            iota_s = cpool.tile([128, 4], I32, tag="iota")
            for dst, src in [(iota_s, iota_d), (bias_s, bias_d),
                             (w1_s, w1_d), (w2_s, w2_d), (wob1_s, wob1_d),
                             (wrg_s, wrg_d), (wob2al_s, wob2al_d),
                             (wob2as_s, wob2as_d), (wob2b_s, wob2b_d),
                             (wfc_s, wfc_d)]:
                nc.sync.dma_start(dst[:], src[:])

            def bias_ap(col):
                return bias_s[:, col : col + 1]

            RELU = mybir.ActivationFunctionType.Relu
            ADD = mybir.AluOpType.add
            MAX = mybir.AluOpType.max
            EQ = mybir.AluOpType.is_equal

            def emit_A(it):
                n0 = it * TILE_N
                st = {}
                obn = ipool.tile([128, 64], I32, tag="obn", name=f"obn{it}")
                src = bass.AP(ob_d, n0 * 16 * obw,
                              [[16 * obw, 128], [128 * 16 * obw, 4], [obw, 16]])
                nc.gpsimd.dma_start(bass.AP(obn[:].tensor, obn[:].offset,
                                            [[64, 128], [16, 4], [1, 16]]), src)
                acn = ipool.tile([128, 4], I32, tag="acn", name=f"acn{it}")
                asrc = bass.AP(ac_d, n0 * obw, [[obw, 128], [128 * obw, 4]])
                nc.gpsimd.dma_start(acn[:], asrc)

                xoh = ohpool.tile([128, 512], mmdt, tag="xoh", name=f"xoh{it}")
                th = xoh[:].tensor
                xo = xoh[:].offset
                nc.vector.tensor_tensor(
                    bass.AP(th, xo, [[512, 128], [128, 4], [4, 16], [1, 4]]),
                    bass.AP(obn[:].tensor, obn[:].offset,
                            [[64, 128], [16, 4], [1, 16], [0, 4]]),
                    bass.AP(iota_s[:].tensor, iota_s[:].offset,
                            [[4, 128], [0, 4], [0, 16], [1, 4]]),
                    EQ,
                )
                nc.vector.tensor_tensor(
                    bass.AP(th, xo + 64, [[512, 128], [128, 4], [1, 4]]),
                    bass.AP(acn[:].tensor, acn[:].offset,
                            [[4, 128], [1, 4], [0, 4]]),
                    bass.AP(iota_s[:].tensor, iota_s[:].offset,
                            [[4, 128], [0, 4], [1, 4]]),
                    EQ,
                )
                if it < 3:
                    # keep the never-read transposed tail finite for the sim
                    nc.gpsimd.memset(
                        bass.AP(th, xo + 68, [[512, 128], [128, 4], [1, 60]]),
                        0.0)
                xt = xtpool.tile([128, 512], mmdt, tag="xt", name=f"xt{it}")
                nc.sync.dma_start_transpose(
                    bass.AP(xt[:].tensor, xt[:].offset,
                            [[512, 128], [128, 4], [1, 128]]),
                    xoh[:])
                st["xt"] = xt
                return st

            def emit_A1(it, st):
                xt = st["xt"]
                x1ps = [pspool.tile([128, 1024], F32, tag="ps",
                                    name=f"x1ps{it}_{_i}") for _i in range(2)]
                for m in range(4):
                    nc.tensor.matmul(
                        x1ps[m // 2][:, (m % 2) * 512 : (m % 2) * 512 + 512],
                        w1_s[:, m * 128 : (m + 1) * 128],
                        xt[0:68, :], start=True, stop=True)
                x1 = [apool.tile([128, 1024], mmdt, tag="x1",
                                 name=f"x1_{it}_{_i}") for _i in range(2)]
                nc.scalar.activation(x1[0][:], x1ps[0][:], RELU, bias=bias_ap(0))
                nc.vector.tensor_scalar(x1[1][:], x1ps[1][:], bias_ap(0), 0.0,
                                        ADD, MAX)
                st["x1"] = x1

            def emit_B1(it, st):
                x2ps = pspool.tile([128, 1024], F32, tag="ps", name=f"x2ps{it}")
                x1 = st["x1"]
                for u in range(2):
                    for h in range(2):
                        nc.tensor.matmul(
                            x2ps[h * 64 : h * 64 + 64, u * 512 : u * 512 + 512],
                            w2_s[:], x1[u][:, h * 512 : h * 512 + 512],
                            start=True, stop=True, skip_group_check=True)
                x2 = apool.tile([128, 1024], mmdt, tag="x2", name=f"x2_{it}")
                nc.scalar.activation(x2[:], x2ps[:], RELU, bias=bias_ap(1))
                st["x2"] = x2

            def emit_B2(it, st):
                x2 = st["x2"]
                nolps = pspool.tile([128, 1024], F32, tag="ps", name=f"nolps{it}")
                for u in range(2):
                    nc.tensor.matmul(nolps[:, u * 512 : u * 512 + 512], wob1_s[:],
                                     x2[:, u * 512 : u * 512 + 512],
                                     start=True, stop=True)
                nol = apool.tile([128, 1024], mmdt, tag="nol", name=f"nol{it}")
                nc.scalar.activation(nol[:], nolps[:], RELU, bias=bias_ap(2))
                st["nol"] = nol

                rgps = pspool.tile([128, 1024], F32, tag="ps", name=f"rgps{it}")
                for u in range(2):
                    nc.tensor.matmul(rgps[:, u * 512 : u * 512 + 512], wrg_s[:],
                                     x2[:, u * 512 : u * 512 + 512],
                                     start=True, stop=True)
                rg = apool.tile([128, 1024], mmdt, tag="rg", name=f"rg{it}")
                nc.vector.tensor_scalar(rg[:], rgps[:], bias_ap(4), 0.0, ADD, MAX)
                st["rg"] = rg

            def emit_C(it, st):
                n0 = it * TILE_N
                xt = st["xt"]
                nol = st["nol"]
                rg = st["rg"]
                late = pspool.tile([128, 1024], F32, tag="ps", name=f"late{it}")
                o8ps = late[:, 0:512]
                nc.tensor.matmul(late[0:32, 0:512], wob2al_s[:],
                                 nol[:, 0:512], start=True, stop=False,
                                 skip_group_check=True)
                nc.tensor.matmul(late[32:64, 0:512], wob2al_s[:],
                                 nol[:, 512:1024], start=True, stop=False,
                                 skip_group_check=True)
                nc.tensor.matmul(late[64:96, 0:512], wob2as_s[:],
                                 nol[:, 0:512], start=True, stop=False,
                                 skip_group_check=True)
                nc.tensor.matmul(late[96:128, 0:512], wob2as_s[:],
                                 nol[:, 512:1024], start=True, stop=False,
                                 tile_position=(0, 96), skip_group_check=True)
                nc.tensor.matmul(o8ps, wob2b_s[:], xt[0:64, :],
                                 start=False, stop=True, skip_group_check=True)
                nc.tensor.matmul(late[0:8, 512:1024], wfc_s[:, 0:8],
                                 rg[:, 0:512], start=True, stop=False,
                                 skip_group_check=True)
                nc.tensor.matmul(late[0:8, 512:1024], wfc_s[:, 8:16],
                                 rg[:, 512:1024], start=False, stop=True,
                                 skip_group_check=True)

                o8 = opool.tile([128, 512], F32, tag="o8", name=f"o8_{it}")
                nc.vector.tensor_scalar(o8[:], o8ps, bias_ap(3), None, ADD)
                fcsb = opool.tile([8, 512], F32, tag="fcsb", name=f"fcsb{it}")
                nc.vector.tensor_scalar(fcsb[:], late[0:8, 512:1024],
                                        bias_s[0:8, 5:6], None, ADD)

                # feature-major stores: loc/scale [64, bc], rg [8, bc]
                nc.sync.dma_start(
                    bass.AP(loc_d, n0, [[bc, 64], [1, 512]]), o8[0:64, :])
                nc.sync.dma_start(
                    bass.AP(scale_d, n0, [[bc, 64], [1, 512]]), o8[64:128, :])
                nc.sync.dma_start(
                    bass.AP(rg_d, n0, [[bc, 8], [1, 512]]), fcsb[:])

            states = {}
            for i in range(nt + 4):
                if i < nt:
                    states[i] = emit_A(i)
                if 0 <= i - 1 < nt:
                    emit_A1(i - 1, states[i - 1])
                if 0 <= i - 2 < nt:
                    emit_B1(i - 2, states[i - 2])
                if 0 <= i - 3 < nt:
                    emit_B2(i - 3, states[i - 3])
                if 0 <= i - 4 < nt:
                    emit_C(i - 4, states[i - 4])
                    del states[i - 4]

    nc.compile()
    return nc


_PROG_CACHE = {}


def _get_program(obw):
    if obw not in _PROG_CACHE:
        _PROG_CACHE[obw] = _build_program(obw)
    return _PROG_CACHE[obw]


def kernel(ob, ac, w_start1, b_start1, w_start2, b_start2, w_ob1, b_ob1,
           w_ob2, b_ob2, w_reward, b_reward, fc_reward_w, fc_reward_b,
           w_go, b_go, fc_go_w, fc_go_b):
    ob = np.asarray(ob)
    ac = np.asarray(ac)
    B = ob.shape[0]
    assert B == B_TOTAL, f"kernel hardcoded for B={B_TOTAL}, got {B}"

    params = dict(w_start1=w_start1, b_start1=b_start1, w_start2=w_start2,
                  b_start2=b_start2, w_ob1=w_ob1, b_ob1=b_ob1, w_ob2=w_ob2,
                  b_ob2=b_ob2, w_reward=w_reward, b_reward=b_reward,
                  fc_reward_w=fc_reward_w, fc_reward_b=fc_reward_b,
                  w_go=w_go, b_go=b_go, fc_go_w=fc_go_w, fc_go_b=fc_go_b)
    params = {k: np.asarray(v, np.float32) for k, v in params.items()}
    mats = _build_weight_mats(params)

    # index tensors: values are 0..3, so int32 is lossless for any int dtype
    obw = 1
    ob_v = np.ascontiguousarray(ob.reshape(B, 16)).astype(np.int32, copy=False)
    ac_v = np.ascontiguousarray(ac).astype(np.int32, copy=False)

    nc = _get_program(obw)

    mmnp = np.float32 if MM_DTYPE == "float32" else None
    def cast_mm(x):
        if MM_DTYPE == "float32":
            return np.asarray(x, np.float32)
        import ml_dtypes
        return np.asarray(x, ml_dtypes.bfloat16)

    consts = {
        "W1": cast_mm(mats["W1"]), "W2": cast_mm(mats["W2"]),
        "Wob1": cast_mm(mats["Wob1"]), "Wob2aL": cast_mm(mats["Wob2aL"]),
        "Wob2aS": cast_mm(mats["Wob2aS"]),
        "Wob2b": cast_mm(mats["Wob2b"]), "Wrg": cast_mm(mats["Wrg"]),
        "Wfc": cast_mm(mats["Wfc"]),
        "biases": mats["biases"],
        "iota4": np.tile(np.arange(4, dtype=np.int32), (128, 1)),
    }

    in_maps = []
    for c in range(NCORES):
        sl = slice(c * BC, (c + 1) * BC)
        in_maps.append({"ob": np.ascontiguousarray(ob_v[sl]),
                        "ac": np.ascontiguousarray(ac_v[c * BC * obw:(c + 1) * BC * obw]),
                        **consts})

    import os
    trace = bool(int(os.environ.get("KERNEL_TRACE", "0")))
    res = run_bass_kernel_spmd(nc, in_maps, core_ids=list(range(NCORES)),
                               trace=trace)
    global _LAST_RESULT
    _LAST_RESULT = res

    # device emits feature-major [64, bc] / [8, bc]; transpose on host
    loc = np.concatenate([res.results[c]["loc"] for c in range(NCORES)], 1)
    scale = np.concatenate([res.results[c]["scale"] for c in range(NCORES)], 1)
    rg = np.concatenate([res.results[c]["rg"] for c in range(NCORES)], 1)

    next_ob_loc = np.ascontiguousarray(loc.T).reshape(B, H, W, N_PIECES)
    next_ob_scale = np.ascontiguousarray(scale.T).reshape(B, H, W, N_PIECES)
    rg = rg.T
    reward_loc = np.ascontiguousarray(rg[:, 0:2])
    reward_scale = np.ascontiguousarray(rg[:, 2:4])
    go_loc = np.ascontiguousarray(rg[:, 4:6])
    go_scale = np.ascontiguousarray(rg[:, 6:8])
    return (next_ob_loc, next_ob_scale, reward_loc, reward_scale,
            go_loc, go_scale)


# revision 41
# speedup vs baseline: 1.4111x; 1.0864x over previous
"""Trainium2 Bass kernel for the DreamWorld dense-CNN model.

Contract: kernel(**inputs) takes the FULL unsharded numpy inputs (as produced
by the reference setup_inputs) and returns the full outputs. Internally the
batch is sharded across 8 NeuronCores (pure data parallel); the small weights
are replicated. All shapes/strategy are hardcoded.
"""

import numpy as np

import concourse.bacc as bacc
import concourse.bass as bass
import concourse.mybir as mybir
import concourse.tile as tile
from concourse.bass_utils import run_bass_kernel_spmd

N_PIECES = 4
N_ACTIONS = 4
H = W = 4
NPIX = H * W
B_TOTAL = 131072
NCORES = 8
BC = B_TOTAL // NCORES          # per-core batch
TILE_N = 512                    # samples per device tile
NT = BC // TILE_N               # tiles per core
NCHUNK = TILE_N // 128          # 128-sample chunks per tile

F32 = mybir.dt.float32
BF16 = mybir.dt.bfloat16
F16 = mybir.dt.float16
I32 = mybir.dt.int32

# matmul/activation compute dtype for the hidden layers ("float32",
# "bfloat16"); outputs always produced/stored in f32.
MM_DTYPE = "bfloat16"


# ---------------------------------------------------------------- host math
def _build_weight_mats(p):
    """Transform reference conv/fc weights into the dense feature-major
    matmul operands used on device. All f32 numpy."""
    w_start1 = p["w_start1"]  # [32, 8, 3, 3]
    b_start1 = p["b_start1"]
    w_start2 = p["w_start2"][:, :, 0, 0]  # [16, 32]
    b_start2 = p["b_start2"]
    w_ob1 = p["w_ob1"][:, :, 0, 0]  # [16, 16]
    b_ob1 = p["b_ob1"]
    w_ob2 = p["w_ob2"][:, :, 0, 0]  # [8, 20]
    b_ob2 = p["b_ob2"]
    w_reward = p["w_reward"][:, :, 0, 0]  # [8, 16]
    b_reward = p["b_reward"]
    fc_r_w = p["fc_reward_w"]  # [4, 128]
    fc_r_b = p["fc_reward_b"]
    w_go = p["w_go"][:, :, 0, 0]
    b_go = p["b_go"]
    fc_g_w = p["fc_go_w"]
    fc_g_b = p["fc_go_b"]

    # conv1 as dense map: input features [68] = 64 ob one-hot (p*4+c) + 4 ac
    # one-hot; output features [512] = p_out*32 + o.
    W1 = np.zeros((68, 512), np.float32)
    for o in range(32):
        for i in range(H):
            for j in range(W):
                p_out = i * W + j
                col = p_out * 32 + o
                for di in (-1, 0, 1):
                    for dj in (-1, 0, 1):
                        ii, jj = i + di, j + dj
                        if 0 <= ii < H and 0 <= jj < W:
                            p_in = ii * W + jj
                            for c in range(N_PIECES):
                                W1[p_in * 4 + c, col] += w_start1[o, c, di + 1, dj + 1]
                            for a in range(N_ACTIONS):
                                W1[64 + a, col] += w_start1[o, 4 + a, di + 1, dj + 1]
    b1vec = np.tile(b_start1, 4).astype(np.float32)  # [128] partition q -> b[q%32]

    # start2: block-diag over 4 pixels per tile; same lhsT for all 4 matmuls
    W2 = np.zeros((128, 64), np.float32)
    for pl in range(4):
        W2[pl * 32 : pl * 32 + 32, pl * 16 : pl * 16 + 16] = w_start2.T
    b2vec = np.tile(b_start2, 8).astype(np.float32)

    # ob1: block-diag over 8 pixels
    Wob1 = np.zeros((128, 128), np.float32)
    for pl in range(8):
        Wob1[pl * 16 : pl * 16 + 16, pl * 16 : pl * 16 + 16] = w_ob1.T
    bob1vec = np.tile(b_ob1, 8).astype(np.float32)

    # ob2: out8 partitions laid out [64 loc | 64 scale], f = p*4 + c within
    # each half. Part a contracts nol (in-ch 0..15), split into loc/scale
    # lhsTs of M=32 (per nol tile u the outputs land at [u*32:u*32+32] and
    # [64+u*32 : 64+u*32+32]). Part b contracts the ob one-hot rows of xT.
    Wob2aL = np.zeros((128, 32), np.float32)
    Wob2aS = np.zeros((128, 32), np.float32)
    for pl in range(8):
        Wob2aL[pl * 16 : pl * 16 + 16, pl * 4 : pl * 4 + 4] = w_ob2[:4, :16].T
        Wob2aS[pl * 16 : pl * 16 + 16, pl * 4 : pl * 4 + 4] = w_ob2[4:, :16].T
    Wob2b = np.zeros((64, 128), np.float32)
    for pp in range(16):
        Wob2b[pp * 4 : pp * 4 + 4, pp * 4 : pp * 4 + 4] = w_ob2[:4, 16:].T
        Wob2b[pp * 4 : pp * 4 + 4, 64 + pp * 4 : 64 + pp * 4 + 4] = w_ob2[4:, 16:].T
    bob2vec = np.concatenate(
        [np.tile(b_ob2[:4], 16), np.tile(b_ob2[4:], 16)]).astype(np.float32)

    # reward+go convs packed: [128 in] -> [64 reward | 64 go]
    Wrg = np.zeros((128, 128), np.float32)
    for pl in range(8):
        Wrg[pl * 16 : pl * 16 + 16, pl * 8 : pl * 8 + 8] = w_reward.T
        Wrg[pl * 16 : pl * 16 + 16, 64 + pl * 8 : 64 + pl * 8 + 8] = w_go.T
    brgvec = np.concatenate([np.tile(b_reward, 8), np.tile(b_go, 8)]).astype(np.float32)

    # fc heads: contraction over rg tile u partitions -> 8 outputs
    Wfc = np.zeros((128, 16), np.float32)  # cols [u*8 : u*8+8]
    for u in range(2):
        for pl in range(8):
            pix = 8 * u + pl
            for o in range(8):
                k = o * 16 + pix
                for j in range(4):
                    Wfc[pl * 8 + o, u * 8 + j] = fc_r_w[j, k]
                    Wfc[64 + pl * 8 + o, u * 8 + 4 + j] = fc_g_w[j, k]
    fcvec = np.zeros(128, np.float32)
    fcvec[:4] = fc_r_b
    fcvec[4:8] = fc_g_b

    # bias matrix [128, 6]: cols = b1, b2, bob1, bob2, brg, fc
    biases = np.stack([b1vec, b2vec, bob1vec, bob2vec, brgvec, fcvec], 1)
    return dict(W1=W1, W2=W2, Wob1=Wob1, Wob2aL=Wob2aL, Wob2aS=Wob2aS,
                Wob2b=Wob2b, Wrg=Wrg, Wfc=Wfc, biases=biases)


# ---------------------------------------------------------------- device IR
def _build_program(ob_words_per_px: int, bc: int = BC):
    """Emit the Bass program. ob_words_per_px: 1 for int32 input, 2 for int64
    (we view the int64 buffer as int32 pairs and read the low words)."""
    nc = bacc.Bacc(None, target_bir_lowering=False, debug=False)
    mmdt = F32 if MM_DTYPE == "float32" else (BF16 if MM_DTYPE == "bfloat16" else F16)
    nt = bc // TILE_N

    obw = ob_words_per_px
    ob_d = nc.dram_tensor("ob", [bc, 16 * obw], I32, kind="ExternalInput")
    ac_d = nc.dram_tensor("ac", [bc * obw], I32, kind="ExternalInput")

    w1_d = nc.dram_tensor("W1", [68, 512], mmdt, kind="ExternalInput")
    w2_d = nc.dram_tensor("W2", [128, 64], mmdt, kind="ExternalInput")
    wob1_d = nc.dram_tensor("Wob1", [128, 128], mmdt, kind="ExternalInput")
    wob2al_d = nc.dram_tensor("Wob2aL", [128, 32], mmdt, kind="ExternalInput")
    wob2as_d = nc.dram_tensor("Wob2aS", [128, 32], mmdt, kind="ExternalInput")
    wob2b_d = nc.dram_tensor("Wob2b", [64, 128], mmdt, kind="ExternalInput")
    wrg_d = nc.dram_tensor("Wrg", [128, 128], mmdt, kind="ExternalInput")
    wfc_d = nc.dram_tensor("Wfc", [128, 16], mmdt, kind="ExternalInput")
    bias_d = nc.dram_tensor("biases", [128, 6], F32, kind="ExternalInput")
    iota_d = nc.dram_tensor("iota4", [128, 4], I32, kind="ExternalInput")

    loc_d = nc.dram_tensor("loc", [64, bc], F32, kind="ExternalOutput")
    scale_d = nc.dram_tensor("scale", [64, bc], F32, kind="ExternalOutput")
    rg_d = nc.dram_tensor("rg", [8, bc], F32, kind="ExternalOutput")

    with tile.TileContext(nc) as tc:
        with (
            tc.tile_pool(name="const", bufs=1) as cpool,
            tc.tile_pool(name="inp", bufs=3) as ipool,
            tc.tile_pool(name="oh", bufs=2) as ohpool,
            tc.tile_pool(name="xts", bufs=3) as xtpool,
            tc.tile_pool(name="act", bufs=4) as apool,
            tc.tile_pool(name="outs", bufs=4) as opool,
            tc.tile_pool(name="ps", bufs=4, space="PSUM") as pspool,
        ):
            # ---- constants
            w1_s = cpool.tile([68, 512], mmdt, tag="w1")
            w2_s = cpool.tile([128, 64], mmdt, tag="w2")
            wob1_s = cpool.tile([128, 128], mmdt, tag="wob1")
            wob2al_s = cpool.tile([128, 32], mmdt, tag="wob2al")
            wob2as_s = cpool.tile([128, 32], mmdt, tag="wob2as")
            wob2b_s = cpool.tile([64, 128], mmdt, tag="wob2b")
            wrg_s = cpool.tile([128, 128], mmdt, tag="wrg")
            wfc_s = cpool.tile([128, 16], mmdt, tag="wfc")
            bias_s = cpool.tile([128, 6], F32, tag="bias")
            iota_s = cpool.tile([128, 4], I32, tag="iota")
            for dst, src in [(iota_s, iota_d), (bias_s, bias_d),
                             (w1_s, w1_d), (w2_s, w2_d), (wob1_s, wob1_d),
                             (wrg_s, wrg_d), (wob2al_s, wob2al_d),
                             (wob2as_s, wob2as_d), (wob2b_s, wob2b_d),
                             (wfc_s, wfc_d)]:
                nc.sync.dma_start(dst[:], src[:])

            def bias_ap(col):
                return bias_s[:, col : col + 1]

            RELU = mybir.ActivationFunctionType.Relu
            ADD = mybir.AluOpType.add
            MAX = mybir.AluOpType.max
            EQ = mybir.AluOpType.is_equal

            def emit_A(it):
                n0 = it * TILE_N
                st = {}
                obn = ipool.tile([128, 64], I32, tag="obn", name=f"obn{it}")
                src = bass.AP(ob_d, n0 * 16 * obw,
                              [[16 * obw, 128], [128 * 16 * obw, 4], [obw, 16]])
                nc.gpsimd.dma_start(bass.AP(obn[:].tensor, obn[:].offset,
                                            [[64, 128], [16, 4], [1, 16]]), src)
                acn = ipool.tile([128, 4], I32, tag="acn", name=f"acn{it}")
                asrc = bass.AP(ac_d, n0 * obw, [[obw, 128], [128 * obw, 4]])
                nc.gpsimd.dma_start(acn[:], asrc)

                if it % 2 == 0:
                    pair_oh[0] = ohpool.tile([128, 1024], mmdt, tag="xoh",
                                             name=f"xoh{it}")
                xoh = pair_oh[0]
                th = xoh[:].tensor
                xo = xoh[:].offset + (it % 2) * 512
                nc.vector.tensor_tensor(
                    bass.AP(th, xo, [[1024, 128], [128, 4], [4, 16], [1, 4]]),
                    bass.AP(obn[:].tensor, obn[:].offset,
                            [[64, 128], [16, 4], [1, 16], [0, 4]]),
                    bass.AP(iota_s[:].tensor, iota_s[:].offset,
                            [[4, 128], [0, 4], [0, 16], [1, 4]]),
                    EQ,
                )
                nc.vector.tensor_tensor(
                    bass.AP(th, xo + 64, [[1024, 128], [128, 4], [1, 4]]),
                    bass.AP(acn[:].tensor, acn[:].offset,
                            [[4, 128], [1, 4], [0, 4]]),
                    bass.AP(iota_s[:].tensor, iota_s[:].offset,
                            [[4, 128], [0, 4], [1, 4]]),
                    EQ,
                )
                if it < 4:
                    # keep the never-read transposed tail finite for the sim
                    nc.gpsimd.memset(
                        bass.AP(th, xo + 68, [[1024, 128], [128, 4], [1, 60]]),
                        0.0)
                if it % 2 == 1 or it == nt - 1:
                    xt2 = xtpool.tile([128, 1024], mmdt, tag="xt",
                                      name=f"xt{it}")
                    nblk = 8 if it % 2 == 1 else 4
                    nc.sync.dma_start_transpose(
                        bass.AP(xt2[:].tensor, xt2[:].offset,
                                [[1024, 128], [128, nblk], [1, 128]]),
                        bass.AP(xoh[:].tensor, xoh[:].offset,
                                [[1024, 128], [1, nblk * 128]]),
                    )
                    if it % 2 == 1:
                        states[it - 1]["xt2"] = xt2
                        states[it - 1]["half"] = 0
                    st["xt2"] = xt2
                    st["half"] = it % 2
                return st

            def emit_A1(it, st):
                xt2 = st["xt2"]
                h0 = st["half"] * 512
                xt = xt2[:, h0 : h0 + 512]
                x1ps = [pspool.tile([128, 1024], F32, tag="ps",
                                    name=f"x1ps{it}_{_i}") for _i in range(2)]
                for m in range(4):
                    nc.tensor.matmul(
                        x1ps[m // 2][:, (m % 2) * 512 : (m % 2) * 512 + 512],
                        w1_s[:, m * 128 : (m + 1) * 128],
                        xt2[0:68, h0 : h0 + 512], start=True, stop=True)
                x1 = [apool.tile([128, 1024], mmdt, tag="x1",
                                 name=f"x1_{it}_{_i}") for _i in range(2)]
                nc.scalar.activation(x1[0][:], x1ps[0][:], RELU, bias=bias_ap(0))
                nc.vector.tensor_scalar(x1[1][:], x1ps[1][:], bias_ap(0), 0.0,
                                        ADD, MAX)
                st["x1"] = x1

            def emit_B1(it, st):
                x2ps = pspool.tile([128, 1024], F32, tag="ps", name=f"x2ps{it}")
                x1 = st["x1"]
                for u in range(2):
                    for h in range(2):
                        nc.tensor.matmul(
                            x2ps[h * 64 : h * 64 + 64, u * 512 : u * 512 + 512],
                            w2_s[:], x1[u][:, h * 512 : h * 512 + 512],
                            start=True, stop=True, skip_group_check=True)
                x2 = apool.tile([128, 1024], mmdt, tag="x2", name=f"x2_{it}")
                nc.scalar.activation(x2[:], x2ps[:], RELU, bias=bias_ap(1))
                st["x2"] = x2

            def emit_B2(it, st):
                x2 = st["x2"]
                nolps = pspool.tile([128, 1024], F32, tag="ps", name=f"nolps{it}")
                for u in range(2):
                    nc.tensor.matmul(nolps[:, u * 512 : u * 512 + 512], wob1_s[:],
                                     x2[:, u * 512 : u * 512 + 512],
                                     start=True, stop=True)
                nol = apool.tile([128, 1024], mmdt, tag="nol", name=f"nol{it}")
                nc.scalar.activation(nol[:], nolps[:], RELU, bias=bias_ap(2))
                st["nol"] = nol

                rgps = pspool.tile([128, 1024], F32, tag="ps", name=f"rgps{it}")
                for u in range(2):
                    nc.tensor.matmul(rgps[:, u * 512 : u * 512 + 512], wrg_s[:],
                                     x2[:, u * 512 : u * 512 + 512],
                                     start=True, stop=True)
                rg = apool.tile([128, 1024], mmdt, tag="rg", name=f"rg{it}")
                nc.vector.tensor_scalar(rg[:], rgps[:], bias_ap(4), 0.0, ADD, MAX)
                st["rg"] = rg

            def emit_C(it, st):
                n0 = it * TILE_N
                xt2 = st["xt2"]
                h0 = st["half"] * 512
                xt = xt2[:, h0 : h0 + 512]
                nol = st["nol"]
                rg = st["rg"]
                late = pspool.tile([128, 1024], F32, tag="ps", name=f"late{it}")
                o8ps = late[:, 0:512]
                nc.tensor.matmul(late[0:32, 0:512], wob2al_s[:],
                                 nol[:, 0:512], start=True, stop=False,
                                 skip_group_check=True)
                nc.tensor.matmul(late[32:64, 0:512], wob2al_s[:],
                                 nol[:, 512:1024], start=True, stop=False,
                                 skip_group_check=True)
                nc.tensor.matmul(late[64:96, 0:512], wob2as_s[:],
                                 nol[:, 0:512], start=True, stop=False,
                                 skip_group_check=True)
                nc.tensor.matmul(late[96:128, 0:512], wob2as_s[:],
                                 nol[:, 512:1024], start=True, stop=False,
                                 tile_position=(0, 96), skip_group_check=True)
                nc.tensor.matmul(o8ps, wob2b_s[:], xt2[0:64, h0 : h0 + 512],
                                 start=False, stop=True, skip_group_check=True)
                nc.tensor.matmul(late[0:8, 512:1024], wfc_s[:, 0:8],
                                 rg[:, 0:512], start=True, stop=False,
                                 skip_group_check=True)
                nc.tensor.matmul(late[0:8, 512:1024], wfc_s[:, 8:16],
                                 rg[:, 512:1024], start=False, stop=True,
                                 skip_group_check=True)

                o8 = opool.tile([128, 512], F32, tag="o8", name=f"o8_{it}")
                nc.vector.tensor_scalar(o8[:], o8ps, bias_ap(3), None, ADD)
                fcsb = opool.tile([8, 512], F32, tag="fcsb", name=f"fcsb{it}")
                nc.vector.tensor_scalar(fcsb[:], late[0:8, 512:1024],
                                        bias_s[0:8, 5:6], None, ADD)

                # feature-major stores: loc/scale [64, bc], rg [8, bc]
                nc.sync.dma_start(
                    bass.AP(loc_d, n0, [[bc, 64], [1, 512]]), o8[0:64, :])
                nc.sync.dma_start(
                    bass.AP(scale_d, n0, [[bc, 64], [1, 512]]), o8[64:128, :])
                nc.sync.dma_start(
                    bass.AP(rg_d, n0, [[bc, 8], [1, 512]]), fcsb[:])

            states = {}
            pair_oh = [None]
            for i in range(nt + 4):
                if i < nt:
                    states[i] = emit_A(i)
                if 0 <= i - 1 < nt:
                    emit_A1(i - 1, states[i - 1])
                if 0 <= i - 2 < nt:
                    emit_B1(i - 2, states[i - 2])
                if 0 <= i - 3 < nt:
                    emit_B2(i - 3, states[i - 3])
                if 0 <= i - 4 < nt:
                    emit_C(i - 4, states[i - 4])
                    del states[i - 4]

    nc.compile()
    return nc


_PROG_CACHE = {}


def _get_program(obw):
    if obw not in _PROG_CACHE:
        _PROG_CACHE[obw] = _build_program(obw)
    return _PROG_CACHE[obw]


def kernel(ob, ac, w_start1, b_start1, w_start2, b_start2, w_ob1, b_ob1,
           w_ob2, b_ob2, w_reward, b_reward, fc_reward_w, fc_reward_b,
           w_go, b_go, fc_go_w, fc_go_b):
    ob = np.asarray(ob)
    ac = np.asarray(ac)
    B = ob.shape[0]
    assert B == B_TOTAL, f"kernel hardcoded for B={B_TOTAL}, got {B}"

    params = dict(w_start1=w_start1, b_start1=b_start1, w_start2=w_start2,
                  b_start2=b_start2, w_ob1=w_ob1, b_ob1=b_ob1, w_ob2=w_ob2,
                  b_ob2=b_ob2, w_reward=w_reward, b_reward=b_reward,
                  fc_reward_w=fc_reward_w, fc_reward_b=fc_reward_b,
                  w_go=w_go, b_go=b_go, fc_go_w=fc_go_w, fc_go_b=fc_go_b)
    params = {k: np.asarray(v, np.float32) for k, v in params.items()}
    mats = _build_weight_mats(params)

    # index tensors: values are 0..3, so int32 is lossless for any int dtype
    obw = 1
    ob_v = np.ascontiguousarray(ob.reshape(B, 16)).astype(np.int32, copy=False)
    ac_v = np.ascontiguousarray(ac).astype(np.int32, copy=False)

    nc = _get_program(obw)

    mmnp = np.float32 if MM_DTYPE == "float32" else None
    def cast_mm(x):
        if MM_DTYPE == "float32":
            return np.asarray(x, np.float32)
        import ml_dtypes
        return np.asarray(x, ml_dtypes.bfloat16)

    consts = {
        "W1": cast_mm(mats["W1"]), "W2": cast_mm(mats["W2"]),
        "Wob1": cast_mm(mats["Wob1"]), "Wob2aL": cast_mm(mats["Wob2aL"]),
        "Wob2aS": cast_mm(mats["Wob2aS"]),
        "Wob2b": cast_mm(mats["Wob2b"]), "Wrg": cast_mm(mats["Wrg"]),
        "Wfc": cast_mm(mats["Wfc"]),
        "biases": mats["biases"],
        "iota4": np.tile(np.arange(4, dtype=np.int32), (128, 1)),
    }

    in_maps = []
    for c in range(NCORES):
        sl = slice(c * BC, (c + 1) * BC)
        in_maps.append({"ob": np.ascontiguousarray(ob_v[sl]),
                        "ac": np.ascontiguousarray(ac_v[c * BC * obw:(c + 1) * BC * obw]),
                        **consts})

    import os
    trace = bool(int(os.environ.get("KERNEL_TRACE", "0")))
    res = run_bass_kernel_spmd(nc, in_maps, core_ids=list(range(NCORES)),
                               trace=trace)
    global _LAST_RESULT
    _LAST_RESULT = res

    # device emits feature-major [64, bc] / [8, bc]; transpose on host
    loc = np.concatenate([res.results[c]["loc"] for c in range(NCORES)], 1)
    scale = np.concatenate([res.results[c]["scale"] for c in range(NCORES)], 1)
    rg = np.concatenate([res.results[c]["rg"] for c in range(NCORES)], 1)

    next_ob_loc = np.ascontiguousarray(loc.T).reshape(B, H, W, N_PIECES)
    next_ob_scale = np.ascontiguousarray(scale.T).reshape(B, H, W, N_PIECES)
    rg = rg.T
    reward_loc = np.ascontiguousarray(rg[:, 0:2])
    reward_scale = np.ascontiguousarray(rg[:, 2:4])
    go_loc = np.ascontiguousarray(rg[:, 4:6])
    go_scale = np.ascontiguousarray(rg[:, 6:8])
    return (next_ob_loc, next_ob_scale, reward_loc, reward_scale,
            go_loc, go_scale)


# revision 42
# speedup vs baseline: 1.4172x; 1.0043x over previous
"""Trainium2 Bass kernel for the DreamWorld dense-CNN model.

Contract: kernel(**inputs) takes the FULL unsharded numpy inputs (as produced
by the reference setup_inputs) and returns the full outputs. Internally the
batch is sharded across 8 NeuronCores (pure data parallel); the small weights
are replicated. All shapes/strategy are hardcoded.
"""

import numpy as np

import concourse.bacc as bacc
import concourse.bass as bass
import concourse.mybir as mybir
import concourse.tile as tile
from concourse.bass_utils import run_bass_kernel_spmd

N_PIECES = 4
N_ACTIONS = 4
H = W = 4
NPIX = H * W
B_TOTAL = 131072
NCORES = 8
BC = B_TOTAL // NCORES          # per-core batch
TILE_N = 512                    # samples per device tile
NT = BC // TILE_N               # tiles per core
NCHUNK = TILE_N // 128          # 128-sample chunks per tile

F32 = mybir.dt.float32
BF16 = mybir.dt.bfloat16
F16 = mybir.dt.float16
I32 = mybir.dt.int32

# matmul/activation compute dtype for the hidden layers ("float32",
# "bfloat16"); outputs always produced/stored in f32.
MM_DTYPE = "bfloat16"


# ---------------------------------------------------------------- host math
def _build_weight_mats(p):
    """Transform reference conv/fc weights into the dense feature-major
    matmul operands used on device. All f32 numpy."""
    w_start1 = p["w_start1"]  # [32, 8, 3, 3]
    b_start1 = p["b_start1"]
    w_start2 = p["w_start2"][:, :, 0, 0]  # [16, 32]
    b_start2 = p["b_start2"]
    w_ob1 = p["w_ob1"][:, :, 0, 0]  # [16, 16]
    b_ob1 = p["b_ob1"]
    w_ob2 = p["w_ob2"][:, :, 0, 0]  # [8, 20]
    b_ob2 = p["b_ob2"]
    w_reward = p["w_reward"][:, :, 0, 0]  # [8, 16]
    b_reward = p["b_reward"]
    fc_r_w = p["fc_reward_w"]  # [4, 128]
    fc_r_b = p["fc_reward_b"]
    w_go = p["w_go"][:, :, 0, 0]
    b_go = p["b_go"]
    fc_g_w = p["fc_go_w"]
    fc_g_b = p["fc_go_b"]

    # conv1 as dense map: input features [68] = 64 ob one-hot (p*4+c) + 4 ac
    # one-hot; output features [512] = p_out*32 + o.
    W1 = np.zeros((68, 512), np.float32)
    for o in range(32):
        for i in range(H):
            for j in range(W):
                p_out = i * W + j
                col = p_out * 32 + o
                for di in (-1, 0, 1):
                    for dj in (-1, 0, 1):
                        ii, jj = i + di, j + dj
                        if 0 <= ii < H and 0 <= jj < W:
                            p_in = ii * W + jj
                            for c in range(N_PIECES):
                                W1[p_in * 4 + c, col] += w_start1[o, c, di + 1, dj + 1]
                            for a in range(N_ACTIONS):
                                W1[64 + a, col] += w_start1[o, 4 + a, di + 1, dj + 1]
    b1vec = np.tile(b_start1, 4).astype(np.float32)  # [128] partition q -> b[q%32]

    # start2: block-diag over 4 pixels per tile; same lhsT for all 4 matmuls
    W2 = np.zeros((128, 64), np.float32)
    for pl in range(4):
        W2[pl * 32 : pl * 32 + 32, pl * 16 : pl * 16 + 16] = w_start2.T
    b2vec = np.tile(b_start2, 8).astype(np.float32)

    # ob1: block-diag over 8 pixels
    Wob1 = np.zeros((128, 128), np.float32)
    for pl in range(8):
        Wob1[pl * 16 : pl * 16 + 16, pl * 16 : pl * 16 + 16] = w_ob1.T
    bob1vec = np.tile(b_ob1, 8).astype(np.float32)

    # ob2: out8 partitions laid out [64 loc | 64 scale], f = p*4 + c within
    # each half. Part a contracts nol (in-ch 0..15), split into loc/scale
    # lhsTs of M=32 (per nol tile u the outputs land at [u*32:u*32+32] and
    # [64+u*32 : 64+u*32+32]). Part b contracts the ob one-hot rows of xT.
    Wob2aL = np.zeros((128, 32), np.float32)
    Wob2aS = np.zeros((128, 32), np.float32)
    for pl in range(8):
        Wob2aL[pl * 16 : pl * 16 + 16, pl * 4 : pl * 4 + 4] = w_ob2[:4, :16].T
        Wob2aS[pl * 16 : pl * 16 + 16, pl * 4 : pl * 4 + 4] = w_ob2[4:, :16].T
    Wob2b = np.zeros((64, 128), np.float32)
    for pp in range(16):
        Wob2b[pp * 4 : pp * 4 + 4, pp * 4 : pp * 4 + 4] = w_ob2[:4, 16:].T
        Wob2b[pp * 4 : pp * 4 + 4, 64 + pp * 4 : 64 + pp * 4 + 4] = w_ob2[4:, 16:].T
    bob2vec = np.concatenate(
        [np.tile(b_ob2[:4], 16), np.tile(b_ob2[4:], 16)]).astype(np.float32)

    # reward+go convs packed: [128 in] -> [64 reward | 64 go]
    Wrg = np.zeros((128, 128), np.float32)
    for pl in range(8):
        Wrg[pl * 16 : pl * 16 + 16, pl * 8 : pl * 8 + 8] = w_reward.T
        Wrg[pl * 16 : pl * 16 + 16, 64 + pl * 8 : 64 + pl * 8 + 8] = w_go.T
    brgvec = np.concatenate([np.tile(b_reward, 8), np.tile(b_go, 8)]).astype(np.float32)

    # fc heads: contraction over rg tile u partitions -> 8 outputs
    Wfc = np.zeros((128, 16), np.float32)  # cols [u*8 : u*8+8]
    for u in range(2):
        for pl in range(8):
            pix = 8 * u + pl
            for o in range(8):
                k = o * 16 + pix
                for j in range(4):
                    Wfc[pl * 8 + o, u * 8 + j] = fc_r_w[j, k]
                    Wfc[64 + pl * 8 + o, u * 8 + 4 + j] = fc_g_w[j, k]
    fcvec = np.zeros(128, np.float32)
    fcvec[:4] = fc_r_b
    fcvec[4:8] = fc_g_b

    # bias matrix [128, 6]: cols = b1, b2, bob1, bob2, brg, fc
    biases = np.stack([b1vec, b2vec, bob1vec, bob2vec, brgvec, fcvec], 1)
    return dict(W1=W1, W2=W2, Wob1=Wob1, Wob2aL=Wob2aL, Wob2aS=Wob2aS,
                Wob2b=Wob2b, Wrg=Wrg, Wfc=Wfc, biases=biases)


# ---------------------------------------------------------------- device IR
def _build_program(ob_words_per_px: int, bc: int = BC):
    """Emit the Bass program. ob_words_per_px: 1 for int32 input, 2 for int64
    (we view the int64 buffer as int32 pairs and read the low words)."""
    nc = bacc.Bacc(None, target_bir_lowering=False, debug=False)
    mmdt = F32 if MM_DTYPE == "float32" else (BF16 if MM_DTYPE == "bfloat16" else F16)
    nt = bc // TILE_N

    obw = ob_words_per_px
    ob_d = nc.dram_tensor("ob", [bc, 16 * obw], I32, kind="ExternalInput")
    ac_d = nc.dram_tensor("ac", [bc * obw], I32, kind="ExternalInput")

    w1_d = nc.dram_tensor("W1", [68, 512], mmdt, kind="ExternalInput")
    w2_d = nc.dram_tensor("W2", [128, 64], mmdt, kind="ExternalInput")
    wob1_d = nc.dram_tensor("Wob1", [128, 128], mmdt, kind="ExternalInput")
    wob2al_d = nc.dram_tensor("Wob2aL", [128, 32], mmdt, kind="ExternalInput")
    wob2as_d = nc.dram_tensor("Wob2aS", [128, 32], mmdt, kind="ExternalInput")
    wob2b_d = nc.dram_tensor("Wob2b", [64, 128], mmdt, kind="ExternalInput")
    wrg_d = nc.dram_tensor("Wrg", [128, 128], mmdt, kind="ExternalInput")
    wfc_d = nc.dram_tensor("Wfc", [128, 16], mmdt, kind="ExternalInput")
    bias_d = nc.dram_tensor("biases", [128, 6], F32, kind="ExternalInput")
    iota_d = nc.dram_tensor("iota4", [128, 4], I32, kind="ExternalInput")

    loc_d = nc.dram_tensor("loc", [64, bc], F32, kind="ExternalOutput")
    scale_d = nc.dram_tensor("scale", [64, bc], F32, kind="ExternalOutput")
    rg_d = nc.dram_tensor("rg", [8, bc], F32, kind="ExternalOutput")

    with tile.TileContext(nc) as tc:
        with (
            tc.tile_pool(name="const", bufs=1) as cpool,
            tc.tile_pool(name="inp", bufs=3) as ipool,
            tc.tile_pool(name="oh", bufs=2) as ohpool,
            tc.tile_pool(name="xts", bufs=3) as xtpool,
            tc.tile_pool(name="act", bufs=4) as apool,
            tc.tile_pool(name="outs", bufs=4) as opool,
            tc.tile_pool(name="ps", bufs=4, space="PSUM") as pspool,
        ):
            # ---- constants
            w1_s = cpool.tile([68, 512], mmdt, tag="w1")
            w2_s = cpool.tile([128, 64], mmdt, tag="w2")
            wob1_s = cpool.tile([128, 128], mmdt, tag="wob1")
            wob2al_s = cpool.tile([128, 32], mmdt, tag="wob2al")
            wob2as_s = cpool.tile([128, 32], mmdt, tag="wob2as")
            wob2b_s = cpool.tile([64, 128], mmdt, tag="wob2b")
            wrg_s = cpool.tile([128, 128], mmdt, tag="wrg")
            wfc_s = cpool.tile([128, 16], mmdt, tag="wfc")
            bias_s = cpool.tile([128, 6], F32, tag="bias")
            iota_s = cpool.tile([128, 4], I32, tag="iota")
            for dst, src in [(iota_s, iota_d), (bias_s, bias_d),
                             (w1_s, w1_d), (w2_s, w2_d), (wob1_s, wob1_d),
                             (wrg_s, wrg_d), (wob2al_s, wob2al_d),
                             (wob2as_s, wob2as_d), (wob2b_s, wob2b_d),
                             (wfc_s, wfc_d)]:
                nc.sync.dma_start(dst[:], src[:])

            def bias_ap(col):
                return bias_s[:, col : col + 1]

            RELU = mybir.ActivationFunctionType.Relu
            ADD = mybir.AluOpType.add
            MAX = mybir.AluOpType.max
            EQ = mybir.AluOpType.is_equal

            def emit_A(it):
                n0 = it * TILE_N
                st = {}
                obn = ipool.tile([128, 64], I32, tag="obn", name=f"obn{it}")
                src = bass.AP(ob_d, n0 * 16 * obw,
                              [[16 * obw, 128], [128 * 16 * obw, 4], [obw, 16]])
                nc.gpsimd.dma_start(bass.AP(obn[:].tensor, obn[:].offset,
                                            [[64, 128], [16, 4], [1, 16]]), src)
                acn = ipool.tile([128, 4], I32, tag="acn", name=f"acn{it}")
                asrc = bass.AP(ac_d, n0 * obw, [[obw, 128], [128 * obw, 4]])
                nc.gpsimd.dma_start(acn[:], asrc)

                if it % 2 == 0:
                    pair_oh[0] = ohpool.tile([128, 1024], mmdt, tag="xoh",
                                             name=f"xoh{it}")
                xoh = pair_oh[0]
                th = xoh[:].tensor
                xo = xoh[:].offset + (it % 2) * 512
                nc.vector.tensor_tensor(
                    bass.AP(th, xo, [[1024, 128], [128, 4], [4, 16], [1, 4]]),
                    bass.AP(obn[:].tensor, obn[:].offset,
                            [[64, 128], [16, 4], [1, 16], [0, 4]]),
                    bass.AP(iota_s[:].tensor, iota_s[:].offset,
                            [[4, 128], [0, 4], [0, 16], [1, 4]]),
                    EQ,
                )
                nc.vector.tensor_tensor(
                    bass.AP(th, xo + 64, [[1024, 128], [128, 4], [1, 4]]),
                    bass.AP(acn[:].tensor, acn[:].offset,
                            [[4, 128], [1, 4], [0, 4]]),
                    bass.AP(iota_s[:].tensor, iota_s[:].offset,
                            [[4, 128], [0, 4], [1, 4]]),
                    EQ,
                )
                if it < 4:
                    # keep the never-read transposed tail finite for the sim
                    nc.gpsimd.memset(
                        bass.AP(th, xo + 68, [[1024, 128], [128, 4], [1, 60]]),
                        0.0)
                if it % 2 == 1 or it == nt - 1:
                    xt2 = xtpool.tile([128, 1024], mmdt, tag="xt",
                                      name=f"xt{it}")
                    nblk = 8 if it % 2 == 1 else 4
                    nc.sync.dma_start_transpose(
                        bass.AP(xt2[:].tensor, xt2[:].offset,
                                [[1024, 128], [128, nblk], [1, 128]]),
                        bass.AP(xoh[:].tensor, xoh[:].offset,
                                [[1024, 128], [1, nblk * 128]]),
                    )
                    if it % 2 == 1:
                        states[it - 1]["xt2"] = xt2
                        states[it - 1]["half"] = 0
                    st["xt2"] = xt2
                    st["half"] = it % 2
                return st

            def emit_A1(it, st):
                xt2 = st["xt2"]
                h0 = st["half"] * 512
                xt = xt2[:, h0 : h0 + 512]
                x1ps = [pspool.tile([128, 1024], F32, tag="ps",
                                    name=f"x1ps{it}_{_i}") for _i in range(2)]
                for m in range(4):
                    nc.tensor.matmul(
                        x1ps[m // 2][:, (m % 2) * 512 : (m % 2) * 512 + 512],
                        w1_s[:, m * 128 : (m + 1) * 128],
                        xt2[0:68, h0 : h0 + 512], start=True, stop=True)
                x1 = [apool.tile([128, 1024], mmdt, tag="x1",
                                 name=f"x1_{it}_{_i}") for _i in range(2)]
                nc.scalar.activation(x1[0][:], x1ps[0][:], RELU, bias=bias_ap(0))
                nc.vector.tensor_scalar(x1[1][:], x1ps[1][:], bias_ap(0), 0.0,
                                        ADD, MAX)
                st["x1"] = x1

            def emit_B1(it, st):
                x2ps = pspool.tile([128, 1024], F32, tag="ps", name=f"x2ps{it}")
                x1 = st["x1"]
                for u in range(2):
                    for h in range(2):
                        nc.tensor.matmul(
                            x2ps[h * 64 : h * 64 + 64, u * 512 : u * 512 + 512],
                            w2_s[:], x1[u][:, h * 512 : h * 512 + 512],
                            start=True, stop=True, skip_group_check=True)
                x2 = apool.tile([128, 1024], mmdt, tag="x2", name=f"x2_{it}")
                nc.scalar.activation(x2[:], x2ps[:], RELU, bias=bias_ap(1))
                st["x2"] = x2

            def emit_B2(it, st):
                x2 = st["x2"]
                nolps = pspool.tile([128, 1024], F32, tag="ps", name=f"nolps{it}")
                for u in range(2):
                    nc.tensor.matmul(nolps[:, u * 512 : u * 512 + 512], wob1_s[:],
                                     x2[:, u * 512 : u * 512 + 512],
                                     start=True, stop=True)
                nol = apool.tile([128, 1024], mmdt, tag="nol", name=f"nol{it}")
                nc.scalar.activation(nol[:], nolps[:], RELU, bias=bias_ap(2))
                st["nol"] = nol

                rgps = pspool.tile([128, 1024], F32, tag="ps", name=f"rgps{it}")
                for u in range(2):
                    nc.tensor.matmul(rgps[:, u * 512 : u * 512 + 512], wrg_s[:],
                                     x2[:, u * 512 : u * 512 + 512],
                                     start=True, stop=True)
                rg = apool.tile([128, 1024], mmdt, tag="rg", name=f"rg{it}")
                nc.vector.tensor_scalar(rg[:], rgps[:], bias_ap(4), 0.0, ADD, MAX)
                st["rg"] = rg

            def emit_C(it, st):
                half = it % 2
                n0 = it * TILE_N
                xt2 = st["xt2"]
                h0 = st["half"] * 512
                xt = xt2[:, h0 : h0 + 512]
                nol = st["nol"]
                rg = st["rg"]
                late = pspool.tile([128, 1024], F32, tag="ps", name=f"late{it}")
                o8ps = late[:, 0:512]
                nc.tensor.matmul(late[0:32, 0:512], wob2al_s[:],
                                 nol[:, 0:512], start=True, stop=False,
                                 skip_group_check=True)
                nc.tensor.matmul(late[32:64, 0:512], wob2al_s[:],
                                 nol[:, 512:1024], start=True, stop=False,
                                 skip_group_check=True)
                nc.tensor.matmul(late[64:96, 0:512], wob2as_s[:],
                                 nol[:, 0:512], start=True, stop=False,
                                 skip_group_check=True)
                nc.tensor.matmul(late[96:128, 0:512], wob2as_s[:],
                                 nol[:, 512:1024], start=True, stop=False,
                                 tile_position=(0, 96), skip_group_check=True)
                nc.tensor.matmul(o8ps, wob2b_s[:], xt2[0:64, h0 : h0 + 512],
                                 start=False, stop=True, skip_group_check=True)
                nc.tensor.matmul(late[0:8, 512:1024], wfc_s[:, 0:8],
                                 rg[:, 0:512], start=True, stop=False,
                                 skip_group_check=True)
                nc.tensor.matmul(late[0:8, 512:1024], wfc_s[:, 8:16],
                                 rg[:, 512:1024], start=False, stop=True,
                                 skip_group_check=True)

                o8 = opool.tile([128, 512], F32, tag="o8", name=f"o8_{it}")
                nc.vector.tensor_scalar(o8[:], o8ps, bias_ap(3), None, ADD)
                fcsb = opool.tile([8, 512], F32, tag="fcsb", name=f"fcsb{it}")
                nc.vector.tensor_scalar(fcsb[:], late[0:8, 512:1024],
                                        bias_s[0:8, 5:6], None, ADD)

                # feature-major stores: loc/scale [64, bc], rg [8, bc]
                nc.sync.dma_start(
                    bass.AP(loc_d, n0, [[bc, 64], [1, 512]]), o8[0:64, :])
                nc.sync.dma_start(
                    bass.AP(scale_d, n0, [[bc, 64], [1, 512]]), o8[64:128, :])
                nc.sync.dma_start(
                    bass.AP(rg_d, n0, [[bc, 8], [1, 512]]), fcsb[:])

            states = {}
            pair_oh = [None]
            pair_out = [None, None]
            for i in range(nt + 4):
                if i < nt:
                    states[i] = emit_A(i)
                if 0 <= i - 1 < nt:
                    emit_A1(i - 1, states[i - 1])
                if 0 <= i - 2 < nt:
                    emit_B1(i - 2, states[i - 2])
                if 0 <= i - 3 < nt:
                    emit_B2(i - 3, states[i - 3])
                if 0 <= i - 4 < nt:
                    emit_C(i - 4, states[i - 4])
                    del states[i - 4]

    nc.compile()
    return nc


_PROG_CACHE = {}


def _get_program(obw):
    if obw not in _PROG_CACHE:
        _PROG_CACHE[obw] = _build_program(obw)
    return _PROG_CACHE[obw]


def kernel(ob, ac, w_start1, b_start1, w_start2, b_start2, w_ob1, b_ob1,
           w_ob2, b_ob2, w_reward, b_reward, fc_reward_w, fc_reward_b,
           w_go, b_go, fc_go_w, fc_go_b):
    ob = np.asarray(ob)
    ac = np.asarray(ac)
    B = ob.shape[0]
    assert B == B_TOTAL, f"kernel hardcoded for B={B_TOTAL}, got {B}"

    params = dict(w_start1=w_start1, b_start1=b_start1, w_start2=w_start2,
                  b_start2=b_start2, w_ob1=w_ob1, b_ob1=b_ob1, w_ob2=w_ob2,
                  b_ob2=b_ob2, w_reward=w_reward, b_reward=b_reward,
                  fc_reward_w=fc_reward_w, fc_reward_b=fc_reward_b,
                  w_go=w_go, b_go=b_go, fc_go_w=fc_go_w, fc_go_b=fc_go_b)
    params = {k: np.asarray(v, np.float32) for k, v in params.items()}
    mats = _build_weight_mats(params)

    # index tensors: values are 0..3, so int32 is lossless for any int dtype
    obw = 1
    ob_v = np.ascontiguousarray(ob.reshape(B, 16)).astype(np.int32, copy=False)
    ac_v = np.ascontiguousarray(ac).astype(np.int32, copy=False)

    nc = _get_program(obw)

    mmnp = np.float32 if MM_DTYPE == "float32" else None
    def cast_mm(x):
        if MM_DTYPE == "float32":
            return np.asarray(x, np.float32)
        import ml_dtypes
        return np.asarray(x, ml_dtypes.bfloat16)

    consts = {
        "W1": cast_mm(mats["W1"]), "W2": cast_mm(mats["W2"]),
        "Wob1": cast_mm(mats["Wob1"]), "Wob2aL": cast_mm(mats["Wob2aL"]),
        "Wob2aS": cast_mm(mats["Wob2aS"]),
        "Wob2b": cast_mm(mats["Wob2b"]), "Wrg": cast_mm(mats["Wrg"]),
        "Wfc": cast_mm(mats["Wfc"]),
        "biases": mats["biases"],
        "iota4": np.tile(np.arange(4, dtype=np.int32), (128, 1)),
    }

    in_maps = []
    for c in range(NCORES):
        sl = slice(c * BC, (c + 1) * BC)
        in_maps.append({"ob": np.ascontiguousarray(ob_v[sl]),
                        "ac": np.ascontiguousarray(ac_v[c * BC * obw:(c + 1) * BC * obw]),
                        **consts})

    import os
    trace = bool(int(os.environ.get("KERNEL_TRACE", "0")))
    res = run_bass_kernel_spmd(nc, in_maps, core_ids=list(range(NCORES)),
                               trace=trace)
    global _LAST_RESULT
    _LAST_RESULT = res

    # device emits feature-major [64, bc] / [8, bc]; transpose on host
    loc = np.concatenate([res.results[c]["loc"] for c in range(NCORES)], 1)
    scale = np.concatenate([res.results[c]["scale"] for c in range(NCORES)], 1)
    rg = np.concatenate([res.results[c]["rg"] for c in range(NCORES)], 1)

    next_ob_loc = np.ascontiguousarray(loc.T).reshape(B, H, W, N_PIECES)
    next_ob_scale = np.ascontiguousarray(scale.T).reshape(B, H, W, N_PIECES)
    rg = rg.T
    reward_loc = np.ascontiguousarray(rg[:, 0:2])
    reward_scale = np.ascontiguousarray(rg[:, 2:4])
    go_loc = np.ascontiguousarray(rg[:, 4:6])
    go_scale = np.ascontiguousarray(rg[:, 6:8])
    return (next_ob_loc, next_ob_scale, reward_loc, reward_scale,
            go_loc, go_scale)
